# revision 13
# baseline (speedup 1.0000x reference)
"""MultiHeadDoubleAttention TRN2 kernel — v5 fp8-DoubleRow convs,
attention half-hidden under the last conv subpass.

Data-parallel over batch: 8 cores x 16 batch each.

Conv: 15x15 hollow-masked conv on an 8x8 grid == 65 shift-taps of
channel matmuls.  Conv matmuls run fp8e4 (TRN E4M3, max 240) in
DoubleRow perf mode: K=256 contraction per instruction (both c-halves
in one pass), 2 multiplies/cell/cycle -> 2x bf16 streaming rate.
Correctness gate is 2e-2; fp8 convs measure ~6e-3 end to end.

Layouts (all per-core):
  activations  [c=128, ch=2, row=8, col=8, b=16] fp8  -- a tap piece's
    rhs is [p, 2, ph, wc*16]: (col,b) merge into one contiguous AP dim.
  weights      [c=128, tap=65, ch=2, o=256] fp8, lhsT per (tap, oh) =
    [p, 2, 128].
  conv psum    [o=128, r=4, c=8, b=16] f32 = one full bank per
    (oh, row-half).

Fixed power-of-2 scales keep fp8 in range: w*2^15, inputs*2^4,
conv1-out*2^6 (folded into the relu-drain ACT).  Conv2 psum carries
2^21; the v-branch drain divides it out (vh unscaled), k/q keep it in
bf16 and the attention exp folds 2^-42 into its scale.

Pipelining:
  - Taps are stored (and DMA'd) in sr-DESCENDING order; passes that
    consume a previous pass's output iterate sr-ASCENDING, so each
    pass's first matmuls depend only on quadrants its producer drained
    mid-stream (per-(oh, row-half) bank hooks fire drains the moment a
    bank's accumulation completes).
  - k/q conv2 runs as two 2-input subpasses (one per output c-half),
    sharing each LDWEIGHTS between the k and q streams.  Heads 0-3 of
    the attention need only the oh=0 half of kh/qh — their full
    scores/exp/AV/normalize/O-transpose chain is interleaved between
    the oh=1 subpass's conv matmuls (PE stays dense and the HAM clock
    never cools); only heads 4-7 + projection remain as a tail.
  - Attention psum lives in the conv pool's tag rings (PSUM is exactly
    8 banks; tags are reused once their conv phase drains).
"""
import sys
sys.path.insert(0, '/opt/trn_rl_repo')
import numpy as np
import ml_dtypes

import concourse.bass as bass
import concourse.bacc as bacc
import concourse.mybir as mybir
import concourse.bass_utils as bass_utils
from concourse.tile import TileContext

F32 = mybir.dt.float32
BF16 = mybir.dt.bfloat16
FP8 = mybir.dt.float8e4
DR = mybir.MatmulPerfMode.DoubleRow

B, D, H, DK = 128, 256, 8, 32
NCORES = 8
BL = B // NCORES          # batch per core
NPIX = 64                 # 8x8
RS = 1.0 / np.sqrt(DK)    # score scale

WS = float(2 ** 15)       # conv weight scale
XS = float(2 ** 4)        # input activation scale
AS = float(2 ** 6)        # conv1-output activation scale
S1 = AS / (WS * XS)       # conv1 psum -> x1 drain scale (2^-13)
F2 = WS * AS              # scale carried by conv2 psum (2^21)
EXPS = RS / (F2 * F2)     # exp scale absorbing kh/qh carry


def hollow_mask():
    m = np.ones((15, 15), np.float32)
    for c in range(5):
        m[1 + c:7, c] = 0; m[8:14 - c, c] = 0
        m[c, 1 + c:7] = 0; m[c, 8:14 - c] = 0
        m[1 + c:7, 14 - c] = 0; m[8:14 - c, 14 - c] = 0
        m[14 - c, 1 + c:7] = 0; m[14 - c, 8:14 - c] = 0
    return m


def tap_schedule():
    """All 65 unmasked taps as (sr, sc, ar0, hr, ac0, wc), stored in
    sr-DESCENDING order (bank0 finishes early when iterated in storage
    order; iterate ASC for the reverse)."""
    m = hollow_mask()
    taps = []
    for di in range(15):
        for dj in range(15):
            if not m[di, dj]:
                continue
            sr, sc = di - 7, dj - 7
            ar0, ar1 = max(0, sr), min(7, 7 + sr)
            ac0, ac1 = max(0, sc), min(7, 7 + sc)
            taps.append((sr, sc, ar0, ar1 - ar0 + 1, ac0, ac1 - ac0 + 1))
    taps.sort(key=lambda e: (-e[0], -(e[3] * e[5])))
    return taps


def tap_pieces(sr, sc, ar0, hr, ac0, wc):
    """Split a tap's output rect at the ir=4 psum-bank boundary.
    Returns list of (bank, ir0_in_bank, ar0, ph, ic0, ac0, wc)."""
    ir0 = ar0 - sr
    ic0 = ac0 - sc
    pieces = []
    lo, hi = ir0, ir0 + hr
    if lo < 4:
        ph = min(hi, 4) - lo
        pieces.append((0, lo, lo + sr, ph, ic0, ac0, wc))
    if hi > 4:
        p0 = max(lo, 4)
        ph = hi - p0
        pieces.append((1, p0 - 4, p0 + sr, ph, ic0, ac0, wc))
    return pieces


TAPS = tap_schedule()
NTAPS = len(TAPS)                       # 65
PIECES = [tap_pieces(*t) for t in TAPS]
# ascending-sr iteration order (indices into storage order):
ASC = sorted(range(NTAPS), key=lambda i: (TAPS[i][0], -TAPS[i][3] * TAPS[i][5]))
# accumulation counts per psum bank half (same for every input / oh):
TOTAL_BK = {0: 0, 1: 0}
for _pl in PIECES:
    for _p in _pl:
        TOTAL_BK[_p[0]] += 1


def chunk_plan(first, rest):
    """Tap chunk sizes [first, rest, rest, ...] covering NTAPS."""
    plan = []
    c0 = 0
    while c0 < NTAPS:
        n = min(first if c0 == 0 else rest, NTAPS - c0)
        plan.append((c0, n))
        c0 += n
    return plan


def load_weights_resident(nc, eng, tile, w_dram, plan):
    """Chunked DMAs of a full conv weight tensor into one resident tile.
    Returns [(c0, n, tile_slice)] for conv_pass."""
    chunks = []
    for c0, n in plan:
        eng.dma_start(tile[:, c0:c0 + n], w_dram[:, c0:c0 + n])
        chunks.append((c0, n, tile[:, c0:c0 + n]))
    return chunks


def load_weights_stream(nc, pool, w_dram, tag, chunk=4):
    """Rotating-chunk DMAs for a use-once conv weight tensor."""
    chunks = []
    for c0, n in chunk_plan(chunk, chunk):
        wt = pool.tile([128, chunk, 2, 256], FP8, tag=f"{tag}wS",
                       name=f"{tag}wS{c0}")
        nc.sync.dma_start(wt[:, :n], w_dram[:, c0:c0 + n])
        chunks.append((c0, n, wt[:, :n]))
    return chunks


def conv_pass(nc, psum_pool, chunks, inputs, tag, ptags, order=None,
              oh_list=(0, 1), interleave=None, interleave_skip=0,
              interleave_every=4, finish=True, on_bank_done=None):
    """One DoubleRow conv layer over `inputs` (list of fp8 tiles
    [128, 2, 8, 8, BL]) for the output-channel halves in `oh_list`.
    Accumulates into psum tiles [128, 4, 8, BL] per (input, o-half,
    pixel-row-half).  `order` permutes tap iteration (storage order if
    None).  `on_bank_done(ii, oh, bk, ps)` fires right after the
    matmul that completes a bank."""
    ps = [{oh: [psum_pool.tile([128, 4, 8, BL], F32,
                               tag=f"{ptags[ii]}{oh}{bk}",
                               name=f"{tag}ps{ii}{oh}{bk}")
               for bk in range(2)] for oh in oh_list}
          for ii in range(len(inputs))]
    items = []
    for c0, n, wt in chunks:
        for tl in range(n):
            items.append((c0 + tl, wt, tl))
    if order is not None:
        items = [items[i] for i in order]
    done = {}
    for j, (ti, wt, tl) in enumerate(items):
        pieces = PIECES[ti]
        for oh in oh_list:
            lhsT = wt[:, tl, :, oh * 128:(oh + 1) * 128]
            for ii, x in enumerate(inputs):
                for (bk, irb, ar0, ph, ic0, ac0, wc) in pieces:
                    key = (ii, oh, bk)
                    cnt = done.get(key, 0)
                    done[key] = cnt + 1
                    rhs = x[:, :, ar0:ar0 + ph, ac0:ac0 + wc, :]
                    out = ps[ii][oh][bk][:, irb:irb + ph,
                                         ic0:ic0 + wc, :]
                    nc.tensor.matmul(
                        out, lhsT, rhs, start=(cnt == 0),
                        stop=(finish and cnt == TOTAL_BK[bk] - 1),
                        perf_mode=DR)
                    if cnt + 1 == TOTAL_BK[bk] and on_bank_done:
                        on_bank_done(ii, oh, bk, ps[ii][oh][bk])
        if (interleave is not None and j >= interleave_skip
                and (j - interleave_skip) % interleave_every
                == interleave_every - 1):
            interleave()
    return ps


def build_kernel():
    """Build the full per-core kernel (same NEFF on all 8 cores)."""
    nc = bacc.Bacc("TRN2", target_bir_lowering=False, debug=False,
                   num_devices=NCORES)
    dt = {}
    for nmm in ("q", "k", "v"):
        dt[f"x{nmm}"] = nc.dram_tensor(f"x{nmm}", [128, 2, 8, 8, BL], FP8,
                                       kind="ExternalInput")
        dt[f"w{nmm}"] = nc.dram_tensor(f"w{nmm}", [128, NTAPS, 2, 256], FP8,
                                       kind="ExternalInput")
        dt[f"bias{nmm}"] = nc.dram_tensor(f"bias{nmm}", [2, 128], F32,
                                          kind="ExternalInput")
    dt["bias2v"] = nc.dram_tensor("bias2v", [2, 128], F32,
                                  kind="ExternalInput")
    dt["wo_t"] = nc.dram_tensor("wo_t", [2, 128, 256], BF16,
                                kind="ExternalInput")
    dt["bo"] = nc.dram_tensor("bo", [1, 256], BF16, kind="ExternalInput")
    dt["bias2k"] = nc.dram_tensor("bias2k", [1, 256], BF16,
                                  kind="ExternalInput")
    dt["ones"] = nc.dram_tensor("ones", [1, 128], BF16, kind="ExternalInput")
    dt["ident"] = nc.dram_tensor("ident", [128, 128], BF16,
                                 kind="ExternalInput")
    dt["ident64"] = nc.dram_tensor("ident64", [128, 64], BF16,
                                   kind="ExternalInput")
    dt["out"] = nc.dram_tensor("out", [8, 128, 256], F32,
                               kind="ExternalOutput")

    with TileContext(nc) as tc:
      with tc.tile_pool(name="persist", bufs=1) as pp:
        # --- persistent SBUF ---
        wv_t = pp.tile([128, NTAPS, 2, 256], FP8, name="wv_t")
        wk_t = pp.tile([128, NTAPS, 2, 256], FP8, name="wk_t")
        bias_t = {}
        for nmm in ("q", "k", "v"):
            bias_t[nmm] = pp.tile([128, 2], F32, name=f"bias{nmm}_t")
        bias2v_t = pp.tile([128, 2], F32, name="bias2v_t")
        ones_t = pp.tile([1, 128], BF16, name="ones_t")
        ones512_t = pp.tile([1, 512], BF16, name="ones512_t")
        biasw_t = pp.tile([1, 256], BF16, name="biasw_t")
        bo_t = pp.tile([1, 256], BF16, name="bo_t")
        ident_t = pp.tile([128, 128], BF16, name="ident_t")
        ident64_t = pp.tile([128, 64], BF16, name="ident64_t")
        ones32_t = pp.tile([128, 32], BF16, name="ones32_t")
        wo_tt = [pp.tile([128, 256], BF16, name=f"wo_tt{h}") for h in range(2)]
        # conv2 outputs [c, b, pix] — live into attention
        hh = {}
        for nmm in ("q", "k", "v"):
            hh[nmm] = [pp.tile([128, BL, NPIX], BF16, name=f"h{nmm}{h}")
                       for h in range(2)]
        E_t = pp.tile([128, BL // 2, H, NPIX], BF16, name="E_t")
        VT = pp.tile([128, BL // 2, H, 33], BF16, name="VT")
        rcp = pp.tile([128, BL // 2, H], F32, name="rcp")
        OAu = pp.tile([128, BL // 2, H, 33], BF16, name="OAu")
        OA = pp.tile([128, BL // 2, 2, 128], BF16, name="OA")
        concat = [pp.tile([128, BL, NPIX], BF16, name=f"concat{h}")
                  for h in range(2)]

        def load_x(pool, nmm, eng):
            xt = pool.tile([128, 2, 8, 8, BL], FP8, name=f"x{nmm}t")
            eng.dma_start(xt[:], dt[f"x{nmm}"].ap())
            return xt

        def relu_hook(x1t, bias):
            # psum [o,4,8,b] -> x1 fp8 quadrant,
            # out = relu(psum*S1 + bias*AS) = 2^6 * relu(y1 + b)
            def f(ii, oh, bk, pst):
                nc.scalar.activation(
                    x1t[:, oh, bk * 4:(bk + 1) * 4, :, :],
                    pst[:],
                    mybir.ActivationFunctionType.Relu,
                    bias=bias[:, oh:oh + 1], scale=S1)
            return f

        def vh_hook(out_t, bias):
            # v-branch conv2 drain: divide out F2, add bv -> vh bf16
            def f(ii, oh, bk, pst):
                nc.scalar.activation(
                    out_t[oh][:, :, bk * 32:(bk + 1) * 32]
                    .rearrange("c b (r q) -> c b r q", r=4),
                    pst[:].rearrange("c r q b -> c b r q"),
                    mybir.ActivationFunctionType.Identity,
                    bias=bias[:, oh:oh + 1], scale=1.0 / F2)
            return f

        def kq_hook(ii, oh, bk, pst):
            # conv2 k/q drain: bias into psum (bk*F2 outer product),
            # then DVE copies (first 2 batches first so the attention's
            # scores unblock early).
            nm = ('k', 'q')[ii]
            nc.tensor.matmul(
                pst[:].rearrange("c r q b -> c (r q b)"),
                biasw_t[:, oh * 128:(oh + 1) * 128],
                ones512_t[:], start=False, stop=True)
            for b0, b1 in ((0, 2), (2, 16)):
                nc.vector.tensor_copy(
                    hh[nm][oh][:, b0:b1, bk * 32:(bk + 1) * 32]
                    .rearrange("c b (r q) -> c b r q", r=4),
                    pst[:, :, :, b0:b1]
                    .rearrange("c r q b -> c b r q"))

        # ================= attention emitters =================
        # Heads hp 0-3 of c-half `oh` live in kh/qh[oh]; a head's whole
        # chain needs only that half.  Psum comes from the conv pool's
        # tag rings: `ptags` names 2 tags for scores and 2 for AV/OT.
        kh, qh = hh['k'], hh['q']

        def emit_scores_h(ps_pool, ptags, b2, oh, salt):
            for hp in range(4):
                h = oh * 4 + hp
                pst = ps_pool.tile([128, NPIX], F32,
                                   tag=ptags[hp % len(ptags)],
                                   name=f"pst{salt}{b2}{h}",
                                   padded_shape=[128, 512])
                for par in range(2):
                    b = 2 * b2 + par
                    nc.tensor.matmul(
                        pst[64 * par:64 * par + 64, :],
                        kh[oh][32 * hp:32 * hp + 32, b, :],
                        qh[oh][32 * hp:32 * hp + 32, b, :],
                        start=True, stop=True,
                        tile_position=(32 * hp, 64 * par))
                nc.scalar.activation(
                    E_t[:, b2, h, :], pst[:],
                    mybir.ActivationFunctionType.Exp, scale=EXPS)

        def emit_av_h(ps_pool, ptags, b2, oh, salt):
            for hp in range(4):
                h = oh * 4 + hp
                pso = ps_pool.tile([128, 33], F32,
                                   tag=ptags[hp % len(ptags)],
                                   name=f"pso{salt}{b2}{h}",
                                   padded_shape=[128, 512])
                for par in range(2):
                    nc.tensor.matmul(
                        pso[64 * par:64 * par + 64, :],
                        E_t[64 * par:64 * par + 64, b2, h, :],
                        VT[64 * par:64 * par + 64, b2, h, :],
                        start=True, stop=True,
                        tile_position=(64 * par, 64 * par))
                nc.vector.tensor_copy(OAu[:, b2, h, :], pso[:])
            nc.vector.reciprocal(rcp[:, b2, oh * 4:oh * 4 + 4],
                                 OAu[:, b2, oh * 4:oh * 4 + 4, 32:33])
            for hp in range(4):
                h = oh * 4 + hp
                nc.vector.tensor_scalar_mul(
                    OA[:, b2, oh, 32 * hp:32 * hp + 32],
                    OAu[:, b2, h, 0:32], rcp[:, b2, h:h + 1])

        def emit_ot(ps_pool, ptag, b2, oh2, salt):
            for par in range(2):
                b = 2 * b2 + par
                pot = ps_pool.tile([128, 64], BF16, tag=ptag,
                                   name=f"pot{salt}{b}{oh2}",
                                   padded_shape=[128, 1024])
                nc.tensor.transpose(
                    pot[:], OA[64 * par:64 * par + 64, b2, oh2, :],
                    ident64_t[64 * par:64 * par + 64, :],
                    tile_position=(64 * par, 0))
                nc.vector.tensor_copy(concat[oh2][:, b, :], pot[:])

        with tc.tile_pool(name="cdata", bufs=1) as cd, \
             tc.tile_pool(name="convps", bufs=1, space="PSUM") as cvp, \
             tc.tile_pool(name="attnsb", bufs=1) as asb:
            # ---- DMA emission in deadline order.  xv rides the scalar
            # queue in parallel with wv's serial sync-queue chunks so
            # P4's first matmuls unblock as early as possible. ----
            xv = load_x(cd, 'v', nc.scalar)
            wv_ch = load_weights_resident(nc, nc.sync, wv_t, dt["wv"].ap(),
                                          chunk_plan(2, 4))
            nc.vector.memset(VT[:, :, :, 32:33], 1.0)
            nc.vector.memset(ones32_t[:], 1.0)
            xq = load_x(cd, 'q', nc.gpsimd)
            xk = load_x(cd, 'k', nc.gpsimd)
            wk_ch = load_weights_resident(nc, nc.sync, wk_t, dt["wk"].ap(),
                                          chunk_plan(8, 8))
            for nmm in ("q", "k", "v"):
                nc.gpsimd.dma_start(bias_t[nmm][:],
                                    dt[f"bias{nmm}"].ap()
                                    .rearrange("h c -> c h"))
            nc.gpsimd.dma_start(bias2v_t[:],
                                dt["bias2v"].ap().rearrange("h c -> c h"))
            nc.gpsimd.dma_start(ones_t[:], dt["ones"][:])
            nc.vector.memset(ones512_t[:], 1.0)
            nc.gpsimd.dma_start(biasw_t[:], dt["bias2k"][:])
            nc.gpsimd.dma_start(bo_t[:], dt["bo"][:])
            nc.scalar.dma_start(ident_t[:], dt["ident"][:])
            nc.scalar.dma_start(ident64_t[:], dt["ident64"][:])
            for h in range(2):
                nc.scalar.dma_start(wo_tt[h][:], dt["wo_t"][h])
            x1 = {}
            for nmm in ("q", "k", "v"):
                x1[nmm] = cd.tile([128, 2, 8, 8, BL], FP8, name=f"x1{nmm}t")

            # ---- P4: v -> v1 (desc order = storage/DMA order) ----
            conv_pass(nc, cvp, wv_ch, [xv], tag="p4", ptags=["cvA"],
                      on_bank_done=relu_hook(x1['v'], bias_t['v']))
            # ---- P5: v1 -> vh (asc order: reads rows 0-3 first) ----
            conv_pass(nc, cvp, wv_ch, [x1['v']], tag="p5", ptags=["cvB"],
                      order=ASC, on_bank_done=vh_hook(hh['v'], bias2v_t))

            # ---- V transposes: vh [c,b,pix] -> VT [kpix, b2, h, dk|1] ----
            # interleaved into P1's matmul stream to keep HAM warm;
            # skipped for the first taps so P5's drain ACTs clear first
            vt_jobs = []
            for b in range(BL):
                par, b2 = b % 2, b // 2
                for oh in range(2):
                    vt_jobs.append((b, par, b2, oh))
            vt_state = {'i': 0}

            def emit_vt(njobs=6):
                for _ in range(njobs):
                    i = vt_state['i']
                    if i >= len(vt_jobs):
                        return
                    vt_state['i'] = i + 1
                    b, par, b2, oh = vt_jobs[i]
                    pvt = cvp.tile([64, 128], BF16, tag=f"cvB0{i % 2}",
                                   name=f"pvt{b}{oh}")
                    nc.tensor.transpose(pvt[:], hh['v'][oh][:, b, :],
                                        ident_t[:])
                    nc.vector.tensor_copy(
                        VT[64 * par:64 * par + 64, b2,
                           oh * 4:(oh + 1) * 4, 0:32],
                        pvt[:].rearrange("k (h d) -> k h d", h=4))

            # ---- P1: q -> q1 (stream wq, storage order) ----
            with tc.tile_pool(name="wstream", bufs=3) as wsp:
                wq_ch = load_weights_stream(nc, wsp, dt["wq"].ap(), tag="q")
                conv_pass(nc, cvp, wq_ch, [xq], tag="p1", ptags=["cvA"],
                          interleave=emit_vt, interleave_skip=8,
                          on_bank_done=relu_hook(x1['q'], bias_t['q']))
                emit_vt(len(vt_jobs))    # any leftovers

            # ---- P2: k -> k1 (desc) ----
            conv_pass(nc, cvp, wk_ch, [xk], tag="p2", ptags=["cvA"],
                      on_bank_done=relu_hook(x1['k'], bias_t['k']))

            # ---- P3a: {k1, q1} -> kh/qh oh=0 half (asc, shared LS) ----
            conv_pass(nc, cvp, wk_ch, [x1['k'], x1['q']], tag="p3a",
                      ptags=["cvA", "cvB"], order=ASC, oh_list=(0,),
                      finish=False, on_bank_done=kq_hook)

            # ---- P3b: oh=1 half, with heads 0-3's attention chain
            # interleaved between its conv matmuls (they only need the
            # oh=0 outputs + VT).  Psum rides the freed oh=0 tag rings.
            attn_jobs = []
            for s in range(BL // 2 + 3):
                if s < BL // 2:
                    attn_jobs.append(('sc', s))
                if 2 <= s < BL // 2 + 2:
                    attn_jobs.append(('av', s - 2))
                if 3 <= s:
                    attn_jobs.append(('ot', s - 3))
            aj_state = {'i': 0}

            def emit_attn_a(njobs=1):
                for _ in range(njobs):
                    i = aj_state['i']
                    if i >= len(attn_jobs):
                        return
                    aj_state['i'] = i + 1
                    op, b2 = attn_jobs[i]
                    if op == 'sc':
                        emit_scores_h(cvp, ["cvA00", "cvA01"], b2, 0, 'a')
                    elif op == 'av':
                        emit_av_h(cvp, ["cvB00", "cvB01"], b2, 0, 'a')
                    else:
                        emit_ot(cvp, ["cvA00", "cvA01"][b2 % 2], b2, 0, 'a')

            conv_pass(nc, cvp, wk_ch, [x1['k'], x1['q']], tag="p3b",
                      ptags=["cvA", "cvB"], order=ASC, oh_list=(1,),
                      finish=False, on_bank_done=kq_hook,
                      interleave=emit_attn_a, interleave_skip=4,
                      interleave_every=2)
            emit_attn_a(len(attn_jobs))    # flush leftovers

            # ---- attention tail: heads 4-7 + projection, pipelined
            # per batch-pair: sc(b2) | av(b2-1) | proj(b2-2).  AV here
            # runs V-stationary: out[dk, qpix] lands [c, pix]-shaped (4
            # heads packed via col groups), a ones-stationary matmul
            # replicates each head's rowsum across its dk partitions,
            # and one reciprocal + one elementwise multiply writes
            # concat directly — no O-transposes.
            def emit_av_b(b2):
                for par in range(2):
                    av = cvp.tile([128, NPIX], F32,
                                  tag=["cvA10", "cvA11"][par],
                                  name=f"avb{b2}{par}",
                                  padded_shape=[128, 512])
                    rs = cvp.tile([128, NPIX], F32,
                                  tag=["cvB10", "cvB11"][par],
                                  name=f"rsb{b2}{par}",
                                  padded_shape=[128, 512])
                    for hp in range(4):
                        h = 4 + hp
                        nc.tensor.matmul(
                            av[32 * hp:32 * hp + 32, :],
                            VT[64 * par:64 * par + 64, b2, h, 0:32],
                            E_t[64 * par:64 * par + 64, b2, h, :],
                            start=True, stop=True,
                            tile_position=(64 * par, 32 * hp))
                        nc.tensor.matmul(
                            rs[32 * hp:32 * hp + 32, :],
                            ones32_t[64 * par:64 * par + 64, :],
                            E_t[64 * par:64 * par + 64, b2, h, :],
                            start=True, stop=True,
                            tile_position=(64 * par, 32 * hp))
                    rr = asb.tile([128, NPIX], F32, tag="rcpB",
                                  name=f"rr{b2}{par}", bufs=2)
                    nc.vector.reciprocal(rr[:], rs[:])
                    nc.vector.tensor_mul(concat[1][:, 2 * b2 + par, :],
                                         av[:], rr[:])

            def emit_proj(blk):
                pspr = cvp.tile([128, 256], F32, tag="cvB00",
                                name=f"pspr{blk}",
                                padded_shape=[128, 512])
                for oh in range(2):
                    nc.tensor.matmul(
                        pspr[:],
                        concat[oh].rearrange("c b p -> c (b p)")
                        [:, blk * 128:(blk + 1) * 128],
                        wo_tt[oh][:], start=(oh == 0), stop=False)
                nc.tensor.matmul(pspr[:], ones_t[:], bo_t[:],
                                 start=False, stop=True)
                osb = asb.tile([128, 256], F32, tag="osb",
                               name=f"osb{blk}", bufs=2)
                nc.vector.tensor_copy(osb[:], pspr[:])
                nc.sync.dma_start(dt["out"][blk], osb[:])

            for step in range(BL // 2 + 2):
                if step < BL // 2:
                    emit_scores_h(cvp, ["cvA00", "cvA01"], step, 1, 'b')
                if 1 <= step < BL // 2 + 1:
                    emit_av_b(step - 1)
                if 2 <= step:
                    emit_proj(step - 2)
    nc.compile()
    return nc


# ---------------------------------------------------------------------------
# Host-side prep
# ---------------------------------------------------------------------------

def prep_weights(w):
    """w: [D, D, 15, 15] OIHW -> [128, NTAPS, 2, 256] fp8e4 laid out
    (c_lo, tap, c-half, o), scaled by WS."""
    wt = np.empty((NTAPS, 2, 128, 256), np.float32)
    for i, (sr, sc, *_r) in enumerate(TAPS):
        # [O, I] -> [I, O] -> [ch, c_lo, O]
        wt[i] = (w[:, :, sr + 7, sc + 7].T * WS).reshape(2, 128, 256)
    wt = wt.transpose(2, 0, 1, 3)   # -> [c_lo, tap, ch, o]
    return np.ascontiguousarray(wt).astype(ml_dtypes.float8_e4m3)


def prep_static(wk, bk, wq, bq, wv, bv, wo, bo):
    """Host-side weight prep shared by all cores."""
    st = {}
    for nmm, w, b in (("q", wq, bq), ("k", wk, bk), ("v", wv, bv)):
        st[f"w{nmm}"] = prep_weights(np.asarray(w, np.float32))
        st[f"bias{nmm}"] = np.ascontiguousarray(
            (np.asarray(b, np.float32) * AS).reshape(2, 128))
    st["bias2v"] = np.ascontiguousarray(
        np.asarray(bv, np.float32).reshape(2, 128))
    st["wo_t"] = np.ascontiguousarray(
        np.asarray(wo, np.float32).T).reshape(2, 128, 256).astype(
        ml_dtypes.bfloat16)
    st["bo"] = np.asarray(bo, np.float32).reshape(1, 256).astype(
        ml_dtypes.bfloat16)
    st["ones"] = np.ones((1, 128), ml_dtypes.bfloat16)
    st["bias2k"] = (np.asarray(bk, np.float32) * F2).reshape(1, 256).astype(
        ml_dtypes.bfloat16)
    st["ident"] = np.eye(128, dtype=ml_dtypes.bfloat16)
    st["ident64"] = np.tile(np.eye(64, dtype=ml_dtypes.bfloat16), (2, 1))
    return st


def prep_core_x(x, core):
    """x: [B, 8, 8, D] -> this core's [128, 2, 8, 8, BL] fp8
    (c_lo, c-half, row, col, b), scaled by XS."""
    xs = np.asarray(x[core * BL:(core + 1) * BL], np.float32) * XS
    xs = xs.transpose(3, 1, 2, 0)                    # [D, r, c, b]
    xs = xs.reshape(2, 128, 8, 8, BL).transpose(1, 0, 2, 3, 4)
    return np.ascontiguousarray(np.clip(xs, -240, 240)).astype(
        ml_dtypes.float8_e4m3)


def make_in_maps(q, k, v, st):
    in_maps = []
    for core in range(NCORES):
        m = dict(st)
        m["xq"] = prep_core_x(q, core)
        m["xk"] = prep_core_x(k, core)
        m["xv"] = prep_core_x(v, core)
        in_maps.append(m)
    return in_maps


def gather_out(results):
    """results: list of dicts with 'out' [8, 128, 256] -> [B, 8, 8, D]."""
    outs = [r["out"].reshape(BL, 8, 8, D) for r in results]
    return np.concatenate(outs, axis=0)


# ---------------------------------------------------------------------------
# Self-contained entry point: kernel(**inputs) -> full [128, 8, 8, 256]
# ---------------------------------------------------------------------------
_NC_CACHE = None


def _get_nc():
    global _NC_CACHE
    if _NC_CACHE is None:
        _NC_CACHE = build_kernel()
    return _NC_CACHE


def kernel(q, k, v, wk, bk, wq, bq, wv, bv, wo, bo):
    nc = _get_nc()
    st = prep_static(wk, bk, wq, bq, wv, bv, wo, bo)
    in_maps = make_in_maps(np.asarray(q), np.asarray(k), np.asarray(v), st)
    res = bass_utils.run_bass_kernel_spmd(
        nc, in_maps, core_ids=list(range(NCORES)))
    return gather_out(res.results)


# revision 14
# speedup vs baseline: 1.0071x; 1.0071x over previous
"""MultiHeadDoubleAttention TRN2 kernel — v5 fp8-DoubleRow convs,
attention half-hidden under the last conv subpass.

Data-parallel over batch: 8 cores x 16 batch each.

Conv: 15x15 hollow-masked conv on an 8x8 grid == 65 shift-taps of
channel matmuls.  Conv matmuls run fp8e4 (TRN E4M3, max 240) in
DoubleRow perf mode: K=256 contraction per instruction (both c-halves
in one pass), 2 multiplies/cell/cycle -> 2x bf16 streaming rate.
Correctness gate is 2e-2; fp8 convs measure ~6e-3 end to end.

Layouts (all per-core):
  activations  [c=128, ch=2, row=8, col=8, b=16] fp8  -- a tap piece's
    rhs is [p, 2, ph, wc*16]: (col,b) merge into one contiguous AP dim.
  weights      [c=128, tap=65, ch=2, o=256] fp8, lhsT per (tap, oh) =
    [p, 2, 128].
  conv psum    [o=128, r=4, c=8, b=16] f32 = one full bank per
    (oh, row-half).

Fixed power-of-2 scales keep fp8 in range: w*2^15, inputs*2^4,
conv1-out*2^6 (folded into the relu-drain ACT).  Conv2 psum carries
2^21; the v-branch drain divides it out (vh unscaled), k/q keep it in
bf16 and the attention exp folds 2^-42 into its scale.

Pipelining:
  - Taps are stored (and DMA'd) in sr-DESCENDING order; passes that
    consume a previous pass's output iterate sr-ASCENDING, so each
    pass's first matmuls depend only on quadrants its producer drained
    mid-stream (per-(oh, row-half) bank hooks fire drains the moment a
    bank's accumulation completes).
  - k/q conv2 runs as two 2-input subpasses (one per output c-half),
    sharing each LDWEIGHTS between the k and q streams.  Heads 0-3 of
    the attention need only the oh=0 half of kh/qh — their full
    scores/exp/AV/normalize/O-transpose chain is interleaved between
    the oh=1 subpass's conv matmuls (PE stays dense and the HAM clock
    never cools); only heads 4-7 + projection remain as a tail.
  - Attention psum lives in the conv pool's tag rings (PSUM is exactly
    8 banks; tags are reused once their conv phase drains).
"""
import sys
sys.path.insert(0, '/opt/trn_rl_repo')
import numpy as np
import ml_dtypes

import concourse.bass as bass
import concourse.bacc as bacc
import concourse.mybir as mybir
import concourse.bass_utils as bass_utils
from concourse.tile import TileContext

F32 = mybir.dt.float32
BF16 = mybir.dt.bfloat16
FP8 = mybir.dt.float8e4
DR = mybir.MatmulPerfMode.DoubleRow

B, D, H, DK = 128, 256, 8, 32
NCORES = 8
BL = B // NCORES          # batch per core
NPIX = 64                 # 8x8
RS = 1.0 / np.sqrt(DK)    # score scale

WS = float(2 ** 15)       # conv weight scale
XS = float(2 ** 4)        # input activation scale
AS = float(2 ** 6)        # conv1-output activation scale
S1 = AS / (WS * XS)       # conv1 psum -> x1 drain scale (2^-13)
F2 = WS * AS              # scale carried by conv2 psum (2^21)
EXPS = RS / (F2 * F2)     # exp scale absorbing kh/qh carry


def hollow_mask():
    m = np.ones((15, 15), np.float32)
    for c in range(5):
        m[1 + c:7, c] = 0; m[8:14 - c, c] = 0
        m[c, 1 + c:7] = 0; m[c, 8:14 - c] = 0
        m[1 + c:7, 14 - c] = 0; m[8:14 - c, 14 - c] = 0
        m[14 - c, 1 + c:7] = 0; m[14 - c, 8:14 - c] = 0
    return m


def tap_schedule():
    """All 65 unmasked taps as (sr, sc, ar0, hr, ac0, wc), stored in
    sr-DESCENDING order (bank0 finishes early when iterated in storage
    order; iterate ASC for the reverse)."""
    m = hollow_mask()
    taps = []
    for di in range(15):
        for dj in range(15):
            if not m[di, dj]:
                continue
            sr, sc = di - 7, dj - 7
            ar0, ar1 = max(0, sr), min(7, 7 + sr)
            ac0, ac1 = max(0, sc), min(7, 7 + sc)
            taps.append((sr, sc, ar0, ar1 - ar0 + 1, ac0, ac1 - ac0 + 1))
    taps.sort(key=lambda e: (-e[0], -(e[3] * e[5])))
    return taps


def tap_pieces(sr, sc, ar0, hr, ac0, wc):
    """Split a tap's output rect at the ir=4 psum-bank boundary.
    Returns list of (bank, ir0_in_bank, ar0, ph, ic0, ac0, wc)."""
    ir0 = ar0 - sr
    ic0 = ac0 - sc
    pieces = []
    lo, hi = ir0, ir0 + hr
    if lo < 4:
        ph = min(hi, 4) - lo
        pieces.append((0, lo, lo + sr, ph, ic0, ac0, wc))
    if hi > 4:
        p0 = max(lo, 4)
        ph = hi - p0
        pieces.append((1, p0 - 4, p0 + sr, ph, ic0, ac0, wc))
    return pieces


TAPS = tap_schedule()
NTAPS = len(TAPS)                       # 65
PIECES = [tap_pieces(*t) for t in TAPS]
# ascending-sr iteration order (indices into storage order):
ASC = sorted(range(NTAPS), key=lambda i: (TAPS[i][0], -TAPS[i][3] * TAPS[i][5]))
# accumulation counts per psum bank half (same for every input / oh):
TOTAL_BK = {0: 0, 1: 0}
for _pl in PIECES:
    for _p in _pl:
        TOTAL_BK[_p[0]] += 1


def chunk_plan(first, rest):
    """Tap chunk sizes [first, rest, rest, ...] covering NTAPS."""
    plan = []
    c0 = 0
    while c0 < NTAPS:
        n = min(first if c0 == 0 else rest, NTAPS - c0)
        plan.append((c0, n))
        c0 += n
    return plan


def load_weights_resident(nc, eng, tile, w_dram, plan):
    """Chunked DMAs of a full conv weight tensor into one resident tile.
    Returns [(c0, n, tile_slice)] for conv_pass."""
    chunks = []
    for c0, n in plan:
        eng.dma_start(tile[:, c0:c0 + n], w_dram[:, c0:c0 + n])
        chunks.append((c0, n, tile[:, c0:c0 + n]))
    return chunks


def load_weights_stream(nc, pool, w_dram, tag, chunk=4):
    """Rotating-chunk DMAs for a use-once conv weight tensor."""
    chunks = []
    for c0, n in chunk_plan(chunk, chunk):
        wt = pool.tile([128, chunk, 2, 256], FP8, tag=f"{tag}wS",
                       name=f"{tag}wS{c0}")
        nc.sync.dma_start(wt[:, :n], w_dram[:, c0:c0 + n])
        chunks.append((c0, n, wt[:, :n]))
    return chunks


def conv_pass(nc, psum_pool, chunks, inputs, tag, ptags, order=None,
              oh_list=(0, 1), interleave=None, interleave_skip=0,
              interleave_every=4, finish=True, on_bank_done=None):
    """One DoubleRow conv layer over `inputs` (list of fp8 tiles
    [128, 2, 8, 8, BL]) for the output-channel halves in `oh_list`.
    Accumulates into psum tiles [128, 4, 8, BL] per (input, o-half,
    pixel-row-half).  `order` permutes tap iteration (storage order if
    None).  `on_bank_done(ii, oh, bk, ps)` fires right after the
    matmul that completes a bank."""
    ps = [{oh: [psum_pool.tile([128, 4, 8, BL], F32,
                               tag=f"{ptags[ii]}{oh}{bk}",
                               name=f"{tag}ps{ii}{oh}{bk}")
               for bk in range(2)] for oh in oh_list}
          for ii in range(len(inputs))]
    items = []
    for c0, n, wt in chunks:
        for tl in range(n):
            items.append((c0 + tl, wt, tl))
    if order is not None:
        items = [items[i] for i in order]
    done = {}
    for j, (ti, wt, tl) in enumerate(items):
        pieces = PIECES[ti]
        for oh in oh_list:
            lhsT = wt[:, tl, :, oh * 128:(oh + 1) * 128]
            for ii, x in enumerate(inputs):
                for (bk, irb, ar0, ph, ic0, ac0, wc) in pieces:
                    key = (ii, oh, bk)
                    cnt = done.get(key, 0)
                    done[key] = cnt + 1
                    rhs = x[:, :, ar0:ar0 + ph, ac0:ac0 + wc, :]
                    out = ps[ii][oh][bk][:, irb:irb + ph,
                                         ic0:ic0 + wc, :]
                    nc.tensor.matmul(
                        out, lhsT, rhs, start=(cnt == 0),
                        stop=(finish and cnt == TOTAL_BK[bk] - 1),
                        perf_mode=DR)
                    if cnt + 1 == TOTAL_BK[bk] and on_bank_done:
                        on_bank_done(ii, oh, bk, ps[ii][oh][bk])
        if (interleave is not None and j >= interleave_skip
                and (j - interleave_skip) % interleave_every
                == interleave_every - 1):
            interleave()
    return ps


def build_kernel():
    """Build the full per-core kernel (same NEFF on all 8 cores)."""
    nc = bacc.Bacc("TRN2", target_bir_lowering=False, debug=False,
                   num_devices=NCORES)
    dt = {}
    for nmm in ("q", "k", "v"):
        dt[f"x{nmm}"] = nc.dram_tensor(f"x{nmm}", [128, 2, 8, 8, BL], FP8,
                                       kind="ExternalInput")
        dt[f"w{nmm}"] = nc.dram_tensor(f"w{nmm}", [128, NTAPS, 2, 256], FP8,
                                       kind="ExternalInput")
        dt[f"bias{nmm}"] = nc.dram_tensor(f"bias{nmm}", [2, 128], F32,
                                          kind="ExternalInput")
    dt["bias2v"] = nc.dram_tensor("bias2v", [2, 128], F32,
                                  kind="ExternalInput")
    dt["wo_t"] = nc.dram_tensor("wo_t", [2, 128, 256], BF16,
                                kind="ExternalInput")
    dt["bo"] = nc.dram_tensor("bo", [1, 256], BF16, kind="ExternalInput")
    dt["bias2k"] = nc.dram_tensor("bias2k", [1, 256], BF16,
                                  kind="ExternalInput")
    dt["ones"] = nc.dram_tensor("ones", [1, 128], BF16, kind="ExternalInput")
    dt["ident"] = nc.dram_tensor("ident", [128, 128], BF16,
                                 kind="ExternalInput")
    dt["ident64"] = nc.dram_tensor("ident64", [128, 64], BF16,
                                   kind="ExternalInput")
    dt["out"] = nc.dram_tensor("out", [8, 128, 256], F32,
                               kind="ExternalOutput")

    with TileContext(nc) as tc:
      with tc.tile_pool(name="persist", bufs=1) as pp:
        # --- persistent SBUF ---
        wv_t = pp.tile([128, NTAPS, 2, 256], FP8, name="wv_t")
        wk_t = pp.tile([128, NTAPS, 2, 256], FP8, name="wk_t")
        bias_t = {}
        for nmm in ("q", "k", "v"):
            bias_t[nmm] = pp.tile([128, 2], F32, name=f"bias{nmm}_t")
        bias2v_t = pp.tile([128, 2], F32, name="bias2v_t")
        ones_t = pp.tile([1, 128], BF16, name="ones_t")
        ones512_t = pp.tile([1, 512], BF16, name="ones512_t")
        biasw_t = pp.tile([1, 256], BF16, name="biasw_t")
        bo_t = pp.tile([1, 256], BF16, name="bo_t")
        ident_t = pp.tile([128, 128], BF16, name="ident_t")
        ident64_t = pp.tile([128, 64], BF16, name="ident64_t")
        ones32_t = pp.tile([128, 32], BF16, name="ones32_t")
        wo_tt = [pp.tile([128, 256], BF16, name=f"wo_tt{h}") for h in range(2)]
        # conv2 outputs [c, b, pix] — live into attention
        hh = {}
        for nmm in ("q", "k", "v"):
            hh[nmm] = [pp.tile([128, BL, NPIX], BF16, name=f"h{nmm}{h}")
                       for h in range(2)]
        E_t = pp.tile([128, BL // 2, H, NPIX], BF16, name="E_t")
        VT = pp.tile([128, BL // 2, H, 33], BF16, name="VT")
        rcp = pp.tile([128, BL // 2, H], F32, name="rcp")
        OAu = pp.tile([128, BL // 2, H, 33], BF16, name="OAu")
        OA = pp.tile([128, BL // 2, 2, 128], BF16, name="OA")
        concat = [pp.tile([128, BL, NPIX], BF16, name=f"concat{h}")
                  for h in range(2)]

        def load_x(pool, nmm, eng):
            xt = pool.tile([128, 2, 8, 8, BL], FP8, name=f"x{nmm}t")
            eng.dma_start(xt[:], dt[f"x{nmm}"].ap())
            return xt

        def relu_hook(x1t, bias):
            # psum [o,4,8,b] -> x1 fp8 quadrant,
            # out = relu(psum*S1 + bias*AS) = 2^6 * relu(y1 + b)
            def f(ii, oh, bk, pst):
                nc.scalar.activation(
                    x1t[:, oh, bk * 4:(bk + 1) * 4, :, :],
                    pst[:],
                    mybir.ActivationFunctionType.Relu,
                    bias=bias[:, oh:oh + 1], scale=S1)
            return f

        def vh_hook(out_t, bias):
            # v-branch conv2 drain: divide out F2, add bv -> vh bf16
            def f(ii, oh, bk, pst):
                nc.scalar.activation(
                    out_t[oh][:, :, bk * 32:(bk + 1) * 32]
                    .rearrange("c b (r q) -> c b r q", r=4),
                    pst[:].rearrange("c r q b -> c b r q"),
                    mybir.ActivationFunctionType.Identity,
                    bias=bias[:, oh:oh + 1], scale=1.0 / F2)
            return f

        def kq_hook(ii, oh, bk, pst):
            # conv2 k/q drain: bias into psum (bk*F2 outer product),
            # then DVE copies (first 2 batches first so the attention's
            # scores unblock early).
            nm = ('k', 'q')[ii]
            nc.tensor.matmul(
                pst[:].rearrange("c r q b -> c (r q b)"),
                biasw_t[:, oh * 128:(oh + 1) * 128],
                ones512_t[:], start=False, stop=True)
            for b0, b1 in ((0, 2), (2, 16)):
                nc.vector.tensor_copy(
                    hh[nm][oh][:, b0:b1, bk * 32:(bk + 1) * 32]
                    .rearrange("c b (r q) -> c b r q", r=4),
                    pst[:, :, :, b0:b1]
                    .rearrange("c r q b -> c b r q"))

        # ================= attention emitters =================
        # Heads hp 0-3 of c-half `oh` live in kh/qh[oh]; a head's whole
        # chain needs only that half.  Psum comes from the conv pool's
        # tag rings: `ptags` names 2 tags for scores and 2 for AV/OT.
        kh, qh = hh['k'], hh['q']

        def emit_scores_h(ps_pool, ptags, b2, oh, salt):
            for hp in range(4):
                h = oh * 4 + hp
                pst = ps_pool.tile([128, NPIX], F32,
                                   tag=ptags[hp % len(ptags)],
                                   name=f"pst{salt}{b2}{h}",
                                   padded_shape=[128, 512])
                for par in range(2):
                    b = 2 * b2 + par
                    nc.tensor.matmul(
                        pst[64 * par:64 * par + 64, :],
                        kh[oh][32 * hp:32 * hp + 32, b, :],
                        qh[oh][32 * hp:32 * hp + 32, b, :],
                        start=True, stop=True,
                        tile_position=(32 * hp, 64 * par))
                nc.scalar.activation(
                    E_t[:, b2, h, :], pst[:],
                    mybir.ActivationFunctionType.Exp, scale=EXPS)

        def emit_av_h(ps_pool, ptags, b2, oh, salt):
            for hp in range(4):
                h = oh * 4 + hp
                pso = ps_pool.tile([128, 33], F32,
                                   tag=ptags[hp % len(ptags)],
                                   name=f"pso{salt}{b2}{h}",
                                   padded_shape=[128, 512])
                for par in range(2):
                    nc.tensor.matmul(
                        pso[64 * par:64 * par + 64, :],
                        E_t[64 * par:64 * par + 64, b2, h, :],
                        VT[64 * par:64 * par + 64, b2, h, :],
                        start=True, stop=True,
                        tile_position=(64 * par, 64 * par))
                nc.vector.tensor_copy(OAu[:, b2, h, :], pso[:])
            nc.vector.reciprocal(rcp[:, b2, oh * 4:oh * 4 + 4],
                                 OAu[:, b2, oh * 4:oh * 4 + 4, 32:33])
            for hp in range(4):
                h = oh * 4 + hp
                nc.vector.tensor_scalar_mul(
                    OA[:, b2, oh, 32 * hp:32 * hp + 32],
                    OAu[:, b2, h, 0:32], rcp[:, b2, h:h + 1])

        def emit_ot(ps_pool, ptag, b2, oh2, salt):
            for par in range(2):
                b = 2 * b2 + par
                pot = ps_pool.tile([128, 64], BF16, tag=ptag,
                                   name=f"pot{salt}{b}{oh2}",
                                   padded_shape=[128, 1024])
                nc.tensor.transpose(
                    pot[:], OA[64 * par:64 * par + 64, b2, oh2, :],
                    ident64_t[64 * par:64 * par + 64, :],
                    tile_position=(64 * par, 0))
                nc.vector.tensor_copy(concat[oh2][:, b, :], pot[:])

        with tc.tile_pool(name="cdata", bufs=1) as cd, \
             tc.tile_pool(name="convps", bufs=1, space="PSUM") as cvp, \
             tc.tile_pool(name="attnsb", bufs=1) as asb:
            # ---- DMA emission in deadline order.  xv rides the scalar
            # queue in parallel with wv's serial sync-queue chunks so
            # P4's first matmuls unblock as early as possible. ----
            xv = load_x(cd, 'v', nc.scalar)
            wv_ch = load_weights_resident(nc, nc.sync, wv_t, dt["wv"].ap(),
                                          chunk_plan(2, 4))
            nc.vector.memset(VT[:, :, :, 32:33], 1.0)
            nc.vector.memset(ones32_t[:], 1.0)
            # ---- PE warm-up: the HAM clock-gate needs ~3.4us of
            # sustained activity to lift the PE from 1.2 to 2.4 GHz.
            # The input DMAs take ~9us to land; burn that dead time on
            # throwaway matmuls so the first real conv runs warm. ----
            scratch = cd.tile([128, 512], BF16, name="scratch")
            nc.vector.memset(scratch[:], 1.0)
            warm_ps = cvp.tile([128, 512], F32, tag="cvB01",
                               name="warm_ps")
            for wi in range(16):
                nc.tensor.matmul(warm_ps[:], scratch[:, 0:128],
                                 scratch[:], start=True, stop=True)
            xq = load_x(cd, 'q', nc.gpsimd)
            xk = load_x(cd, 'k', nc.gpsimd)
            wk_ch = load_weights_resident(nc, nc.sync, wk_t, dt["wk"].ap(),
                                          chunk_plan(8, 8))
            for nmm in ("q", "k", "v"):
                nc.gpsimd.dma_start(bias_t[nmm][:],
                                    dt[f"bias{nmm}"].ap()
                                    .rearrange("h c -> c h"))
            nc.gpsimd.dma_start(bias2v_t[:],
                                dt["bias2v"].ap().rearrange("h c -> c h"))
            nc.gpsimd.dma_start(ones_t[:], dt["ones"][:])
            nc.vector.memset(ones512_t[:], 1.0)
            nc.gpsimd.dma_start(biasw_t[:], dt["bias2k"][:])
            nc.gpsimd.dma_start(bo_t[:], dt["bo"][:])
            nc.scalar.dma_start(ident_t[:], dt["ident"][:])
            nc.scalar.dma_start(ident64_t[:], dt["ident64"][:])
            for h in range(2):
                nc.scalar.dma_start(wo_tt[h][:], dt["wo_t"][h])
            x1 = {}
            for nmm in ("q", "k", "v"):
                x1[nmm] = cd.tile([128, 2, 8, 8, BL], FP8, name=f"x1{nmm}t")

            # ---- P4: v -> v1 (desc order = storage/DMA order) ----
            conv_pass(nc, cvp, wv_ch, [xv], tag="p4", ptags=["cvA"],
                      on_bank_done=relu_hook(x1['v'], bias_t['v']))
            # ---- P5: v1 -> vh (asc order: reads rows 0-3 first) ----
            conv_pass(nc, cvp, wv_ch, [x1['v']], tag="p5", ptags=["cvB"],
                      order=ASC, on_bank_done=vh_hook(hh['v'], bias2v_t))

            # ---- V transposes: vh [c,b,pix] -> VT [kpix, b2, h, dk|1] ----
            # interleaved into P1's matmul stream to keep HAM warm;
            # skipped for the first taps so P5's drain ACTs clear first
            vt_jobs = []
            for b in range(BL):
                par, b2 = b % 2, b // 2
                for oh in range(2):
                    vt_jobs.append((b, par, b2, oh))
            vt_state = {'i': 0}

            def emit_vt(njobs=6):
                for _ in range(njobs):
                    i = vt_state['i']
                    if i >= len(vt_jobs):
                        return
                    vt_state['i'] = i + 1
                    b, par, b2, oh = vt_jobs[i]
                    pvt = cvp.tile([64, 128], BF16, tag=f"cvB0{i % 2}",
                                   name=f"pvt{b}{oh}")
                    nc.tensor.transpose(pvt[:], hh['v'][oh][:, b, :],
                                        ident_t[:])
                    nc.vector.tensor_copy(
                        VT[64 * par:64 * par + 64, b2,
                           oh * 4:(oh + 1) * 4, 0:32],
                        pvt[:].rearrange("k (h d) -> k h d", h=4))

            # ---- P1: q -> q1 (stream wq, storage order) ----
            with tc.tile_pool(name="wstream", bufs=3) as wsp:
                wq_ch = load_weights_stream(nc, wsp, dt["wq"].ap(), tag="q")
                conv_pass(nc, cvp, wq_ch, [xq], tag="p1", ptags=["cvA"],
                          interleave=emit_vt, interleave_skip=8,
                          on_bank_done=relu_hook(x1['q'], bias_t['q']))
                emit_vt(len(vt_jobs))    # any leftovers

            # ---- P2: k -> k1 (desc) ----
            conv_pass(nc, cvp, wk_ch, [xk], tag="p2", ptags=["cvA"],
                      on_bank_done=relu_hook(x1['k'], bias_t['k']))

            # ---- P3a: {k1, q1} -> kh/qh oh=0 half (asc, shared LS) ----
            conv_pass(nc, cvp, wk_ch, [x1['k'], x1['q']], tag="p3a",
                      ptags=["cvA", "cvB"], order=ASC, oh_list=(0,),
                      finish=False, on_bank_done=kq_hook)

            # ---- P3b: oh=1 half, with heads 0-3's attention chain
            # interleaved between its conv matmuls (they only need the
            # oh=0 outputs + VT).  Psum rides the freed oh=0 tag rings.
            attn_jobs = []
            for s in range(BL // 2 + 3):
                if s < BL // 2:
                    attn_jobs.append(('sc', s))
                if 2 <= s < BL // 2 + 2:
                    attn_jobs.append(('av', s - 2))
                if 3 <= s:
                    attn_jobs.append(('ot', s - 3))
            aj_state = {'i': 0}

            def emit_attn_a(njobs=1):
                for _ in range(njobs):
                    i = aj_state['i']
                    if i >= len(attn_jobs):
                        return
                    aj_state['i'] = i + 1
                    op, b2 = attn_jobs[i]
                    if op == 'sc':
                        emit_scores_h(cvp, ["cvA00", "cvA01"], b2, 0, 'a')
                    elif op == 'av':
                        emit_av_h(cvp, ["cvB00", "cvB01"], b2, 0, 'a')
                    else:
                        emit_ot(cvp, ["cvA00", "cvA01"][b2 % 2], b2, 0, 'a')

            conv_pass(nc, cvp, wk_ch, [x1['k'], x1['q']], tag="p3b",
                      ptags=["cvA", "cvB"], order=ASC, oh_list=(1,),
                      finish=False, on_bank_done=kq_hook,
                      interleave=emit_attn_a, interleave_skip=4,
                      interleave_every=2)
            emit_attn_a(len(attn_jobs))    # flush leftovers

            # ---- attention tail: heads 4-7 + projection, pipelined
            # per batch-pair: sc(b2) | av(b2-1) | proj(b2-2).  AV here
            # runs V-stationary: out[dk, qpix] lands [c, pix]-shaped (4
            # heads packed via col groups), a ones-stationary matmul
            # replicates each head's rowsum across its dk partitions,
            # and one reciprocal + one elementwise multiply writes
            # concat directly — no O-transposes.
            def emit_av_b(b2):
                for par in range(2):
                    av = cvp.tile([128, NPIX], F32,
                                  tag=["cvA10", "cvA11"][par],
                                  name=f"avb{b2}{par}",
                                  padded_shape=[128, 512])
                    rs = cvp.tile([128, NPIX], F32,
                                  tag=["cvB10", "cvB11"][par],
                                  name=f"rsb{b2}{par}",
                                  padded_shape=[128, 512])
                    for hp in range(4):
                        h = 4 + hp
                        nc.tensor.matmul(
                            av[32 * hp:32 * hp + 32, :],
                            VT[64 * par:64 * par + 64, b2, h, 0:32],
                            E_t[64 * par:64 * par + 64, b2, h, :],
                            start=True, stop=True,
                            tile_position=(64 * par, 32 * hp))
                        nc.tensor.matmul(
                            rs[32 * hp:32 * hp + 32, :],
                            ones32_t[64 * par:64 * par + 64, :],
                            E_t[64 * par:64 * par + 64, b2, h, :],
                            start=True, stop=True,
                            tile_position=(64 * par, 32 * hp))
                    rr = asb.tile([128, NPIX], F32, tag="rcpB",
                                  name=f"rr{b2}{par}", bufs=2)
                    nc.vector.reciprocal(rr[:], rs[:])
                    nc.vector.tensor_mul(concat[1][:, 2 * b2 + par, :],
                                         av[:], rr[:])

            def emit_proj(blk):
                pspr = cvp.tile([128, 256], F32, tag="cvB00",
                                name=f"pspr{blk}",
                                padded_shape=[128, 512])
                for oh in range(2):
                    nc.tensor.matmul(
                        pspr[:],
                        concat[oh].rearrange("c b p -> c (b p)")
                        [:, blk * 128:(blk + 1) * 128],
                        wo_tt[oh][:], start=(oh == 0), stop=False)
                nc.tensor.matmul(pspr[:], ones_t[:], bo_t[:],
                                 start=False, stop=True)
                osb = asb.tile([128, 256], F32, tag="osb",
                               name=f"osb{blk}", bufs=2)
                nc.vector.tensor_copy(osb[:], pspr[:])
                nc.sync.dma_start(dt["out"][blk], osb[:])

            for step in range(BL // 2 + 2):
                if step < BL // 2:
                    emit_scores_h(cvp, ["cvA00", "cvA01"], step, 1, 'b')
                if 1 <= step < BL // 2 + 1:
                    emit_av_b(step - 1)
                if 2 <= step:
                    emit_proj(step - 2)
    nc.compile()
    return nc


# ---------------------------------------------------------------------------
# Host-side prep
# ---------------------------------------------------------------------------

def prep_weights(w):
    """w: [D, D, 15, 15] OIHW -> [128, NTAPS, 2, 256] fp8e4 laid out
    (c_lo, tap, c-half, o), scaled by WS."""
    wt = np.empty((NTAPS, 2, 128, 256), np.float32)
    for i, (sr, sc, *_r) in enumerate(TAPS):
        # [O, I] -> [I, O] -> [ch, c_lo, O]
        wt[i] = (w[:, :, sr + 7, sc + 7].T * WS).reshape(2, 128, 256)
    wt = wt.transpose(2, 0, 1, 3)   # -> [c_lo, tap, ch, o]
    return np.ascontiguousarray(wt).astype(ml_dtypes.float8_e4m3)


def prep_static(wk, bk, wq, bq, wv, bv, wo, bo):
    """Host-side weight prep shared by all cores."""
    st = {}
    for nmm, w, b in (("q", wq, bq), ("k", wk, bk), ("v", wv, bv)):
        st[f"w{nmm}"] = prep_weights(np.asarray(w, np.float32))
        st[f"bias{nmm}"] = np.ascontiguousarray(
            (np.asarray(b, np.float32) * AS).reshape(2, 128))
    st["bias2v"] = np.ascontiguousarray(
        np.asarray(bv, np.float32).reshape(2, 128))
    st["wo_t"] = np.ascontiguousarray(
        np.asarray(wo, np.float32).T).reshape(2, 128, 256).astype(
        ml_dtypes.bfloat16)
    st["bo"] = np.asarray(bo, np.float32).reshape(1, 256).astype(
        ml_dtypes.bfloat16)
    st["ones"] = np.ones((1, 128), ml_dtypes.bfloat16)
    st["bias2k"] = (np.asarray(bk, np.float32) * F2).reshape(1, 256).astype(
        ml_dtypes.bfloat16)
    st["ident"] = np.eye(128, dtype=ml_dtypes.bfloat16)
    st["ident64"] = np.tile(np.eye(64, dtype=ml_dtypes.bfloat16), (2, 1))
    return st


def prep_core_x(x, core):
    """x: [B, 8, 8, D] -> this core's [128, 2, 8, 8, BL] fp8
    (c_lo, c-half, row, col, b), scaled by XS."""
    xs = np.asarray(x[core * BL:(core + 1) * BL], np.float32) * XS
    xs = xs.transpose(3, 1, 2, 0)                    # [D, r, c, b]
    xs = xs.reshape(2, 128, 8, 8, BL).transpose(1, 0, 2, 3, 4)
    return np.ascontiguousarray(np.clip(xs, -240, 240)).astype(
        ml_dtypes.float8_e4m3)


def make_in_maps(q, k, v, st):
    in_maps = []
    for core in range(NCORES):
        m = dict(st)
        m["xq"] = prep_core_x(q, core)
        m["xk"] = prep_core_x(k, core)
        m["xv"] = prep_core_x(v, core)
        in_maps.append(m)
    return in_maps


def gather_out(results):
    """results: list of dicts with 'out' [8, 128, 256] -> [B, 8, 8, D]."""
    outs = [r["out"].reshape(BL, 8, 8, D) for r in results]
    return np.concatenate(outs, axis=0)


# ---------------------------------------------------------------------------
# Self-contained entry point: kernel(**inputs) -> full [128, 8, 8, 256]
# ---------------------------------------------------------------------------
_NC_CACHE = None


def _get_nc():
    global _NC_CACHE
    if _NC_CACHE is None:
        _NC_CACHE = build_kernel()
    return _NC_CACHE


def kernel(q, k, v, wk, bk, wq, bq, wv, bv, wo, bo):
    nc = _get_nc()
    st = prep_static(wk, bk, wq, bq, wv, bv, wo, bo)
    in_maps = make_in_maps(np.asarray(q), np.asarray(k), np.asarray(v), st)
    res = bass_utils.run_bass_kernel_spmd(
        nc, in_maps, core_ids=list(range(NCORES)))
    return gather_out(res.results)


# revision 17
# speedup vs baseline: 1.0081x; 1.0009x over previous
"""MultiHeadDoubleAttention TRN2 kernel — v5 fp8-DoubleRow convs,
attention half-hidden under the last conv subpass.

Data-parallel over batch: 8 cores x 16 batch each.

Conv: 15x15 hollow-masked conv on an 8x8 grid == 65 shift-taps of
channel matmuls.  Conv matmuls run fp8e4 (TRN E4M3, max 240) in
DoubleRow perf mode: K=256 contraction per instruction (both c-halves
in one pass), 2 multiplies/cell/cycle -> 2x bf16 streaming rate.
Correctness gate is 2e-2; fp8 convs measure ~6e-3 end to end.

Layouts (all per-core):
  activations  [c=128, ch=2, row=8, col=8, b=16] fp8  -- a tap piece's
    rhs is [p, 2, ph, wc*16]: (col,b) merge into one contiguous AP dim.
  weights      [c=128, tap=65, ch=2, o=256] fp8, lhsT per (tap, oh) =
    [p, 2, 128].
  conv psum    [o=128, r=4, c=8, b=16] f32 = one full bank per
    (oh, row-half).

Fixed power-of-2 scales keep fp8 in range: w*2^15, inputs*2^4,
conv1-out*2^6 (folded into the relu-drain ACT).  Conv2 psum carries
2^21; the v-branch drain divides it out (vh unscaled), k/q keep it in
bf16 and the attention exp folds 2^-42 into its scale.

Pipelining:
  - Taps are stored (and DMA'd) in sr-DESCENDING order; passes that
    consume a previous pass's output iterate sr-ASCENDING, so each
    pass's first matmuls depend only on quadrants its producer drained
    mid-stream (per-(oh, row-half) bank hooks fire drains the moment a
    bank's accumulation completes).
  - k/q conv2 runs as two 2-input subpasses (one per output c-half),
    sharing each LDWEIGHTS between the k and q streams.  Heads 0-3 of
    the attention need only the oh=0 half of kh/qh — their full
    scores/exp/AV/normalize/O-transpose chain is interleaved between
    the oh=1 subpass's conv matmuls (PE stays dense and the HAM clock
    never cools); only heads 4-7 + projection remain as a tail.
  - Attention psum lives in the conv pool's tag rings (PSUM is exactly
    8 banks; tags are reused once their conv phase drains).
"""
import sys
sys.path.insert(0, '/opt/trn_rl_repo')
import numpy as np
import ml_dtypes

import concourse.bass as bass
import concourse.bacc as bacc
import concourse.mybir as mybir
import concourse.bass_utils as bass_utils
from concourse.tile import TileContext

F32 = mybir.dt.float32
BF16 = mybir.dt.bfloat16
FP8 = mybir.dt.float8e4
DR = mybir.MatmulPerfMode.DoubleRow

B, D, H, DK = 128, 256, 8, 32
NCORES = 8
BL = B // NCORES          # batch per core
NPIX = 64                 # 8x8
RS = 1.0 / np.sqrt(DK)    # score scale

WS = float(2 ** 15)       # conv weight scale
XS = float(2 ** 4)        # input activation scale
AS = float(2 ** 6)        # conv1-output activation scale
S1 = AS / (WS * XS)       # conv1 psum -> x1 drain scale (2^-13)
F2 = WS * AS              # scale carried by conv2 psum (2^21)
EXPS = RS / (F2 * F2)     # exp scale absorbing kh/qh carry


def hollow_mask():
    m = np.ones((15, 15), np.float32)
    for c in range(5):
        m[1 + c:7, c] = 0; m[8:14 - c, c] = 0
        m[c, 1 + c:7] = 0; m[c, 8:14 - c] = 0
        m[1 + c:7, 14 - c] = 0; m[8:14 - c, 14 - c] = 0
        m[14 - c, 1 + c:7] = 0; m[14 - c, 8:14 - c] = 0
    return m


def tap_schedule():
    """All 65 unmasked taps as (sr, sc, ar0, hr, ac0, wc), stored in
    sr-DESCENDING order (bank0 finishes early when iterated in storage
    order; iterate ASC for the reverse)."""
    m = hollow_mask()
    taps = []
    for di in range(15):
        for dj in range(15):
            if not m[di, dj]:
                continue
            sr, sc = di - 7, dj - 7
            ar0, ar1 = max(0, sr), min(7, 7 + sr)
            ac0, ac1 = max(0, sc), min(7, 7 + sc)
            taps.append((sr, sc, ar0, ar1 - ar0 + 1, ac0, ac1 - ac0 + 1))
    taps.sort(key=lambda e: (-e[0], -(e[3] * e[5])))
    return taps


def tap_pieces(sr, sc, ar0, hr, ac0, wc):
    """Split a tap's output rect at the ir=4 psum-bank boundary.
    Returns list of (bank, ir0_in_bank, ar0, ph, ic0, ac0, wc)."""
    ir0 = ar0 - sr
    ic0 = ac0 - sc
    pieces = []
    lo, hi = ir0, ir0 + hr
    if lo < 4:
        ph = min(hi, 4) - lo
        pieces.append((0, lo, lo + sr, ph, ic0, ac0, wc))
    if hi > 4:
        p0 = max(lo, 4)
        ph = hi - p0
        pieces.append((1, p0 - 4, p0 + sr, ph, ic0, ac0, wc))
    return pieces


TAPS = tap_schedule()
NTAPS = len(TAPS)                       # 65
PIECES = [tap_pieces(*t) for t in TAPS]
# ascending-sr iteration order (indices into storage order):
ASC = sorted(range(NTAPS), key=lambda i: (TAPS[i][0], -TAPS[i][3] * TAPS[i][5]))
# accumulation counts per psum bank half (same for every input / oh):
TOTAL_BK = {0: 0, 1: 0}
for _pl in PIECES:
    for _p in _pl:
        TOTAL_BK[_p[0]] += 1


def chunk_plan(first, rest):
    """Tap chunk sizes [first, rest, rest, ...] covering NTAPS."""
    plan = []
    c0 = 0
    while c0 < NTAPS:
        n = min(first if c0 == 0 else rest, NTAPS - c0)
        plan.append((c0, n))
        c0 += n
    return plan


def load_weights_resident(nc, eng, tile, w_dram, plan, par_head=0):
    """Chunked DMAs of a full conv weight tensor into one resident tile.
    The first `par_head` chunk triggers round-robin over the three
    DMA-capable queues so their transfers land in parallel (per-chunk
    tile deps make arrival order irrelevant); the rest ride `eng`.
    Returns [(c0, n, tile_slice)] for conv_pass."""
    chunks = []
    head_engines = [nc.sync, nc.scalar, nc.gpsimd]
    for i, (c0, n) in enumerate(plan):
        e = head_engines[i % 3] if i < par_head else eng
        e.dma_start(tile[:, c0:c0 + n], w_dram[:, c0:c0 + n])
        chunks.append((c0, n, tile[:, c0:c0 + n]))
    return chunks


def load_weights_stream(nc, pool, w_dram, tag, chunk=4):
    """Rotating-chunk DMAs for a use-once conv weight tensor."""
    chunks = []
    for c0, n in chunk_plan(chunk, chunk):
        wt = pool.tile([128, chunk, 2, 256], FP8, tag=f"{tag}wS",
                       name=f"{tag}wS{c0}")
        nc.sync.dma_start(wt[:, :n], w_dram[:, c0:c0 + n])
        chunks.append((c0, n, wt[:, :n]))
    return chunks


def conv_pass(nc, psum_pool, chunks, inputs, tag, ptags, order=None,
              oh_list=(0, 1), interleave=None, interleave_skip=0,
              interleave_every=4, finish=True, on_bank_done=None):
    """One DoubleRow conv layer over `inputs` (list of fp8 tiles
    [128, 2, 8, 8, BL]) for the output-channel halves in `oh_list`.
    Accumulates into psum tiles [128, 4, 8, BL] per (input, o-half,
    pixel-row-half).  `order` permutes tap iteration (storage order if
    None).  `on_bank_done(ii, oh, bk, ps)` fires right after the
    matmul that completes a bank."""
    ps = [{oh: [psum_pool.tile([128, 4, 8, BL], F32,
                               tag=f"{ptags[ii]}{oh}{bk}",
                               name=f"{tag}ps{ii}{oh}{bk}")
               for bk in range(2)] for oh in oh_list}
          for ii in range(len(inputs))]
    items = []
    for c0, n, wt in chunks:
        for tl in range(n):
            items.append((c0 + tl, wt, tl))
    if order is not None:
        items = [items[i] for i in order]
    done = {}
    for j, (ti, wt, tl) in enumerate(items):
        pieces = PIECES[ti]
        for oh in oh_list:
            lhsT = wt[:, tl, :, oh * 128:(oh + 1) * 128]
            for ii, x in enumerate(inputs):
                for (bk, irb, ar0, ph, ic0, ac0, wc) in pieces:
                    key = (ii, oh, bk)
                    cnt = done.get(key, 0)
                    done[key] = cnt + 1
                    rhs = x[:, :, ar0:ar0 + ph, ac0:ac0 + wc, :]
                    out = ps[ii][oh][bk][:, irb:irb + ph,
                                         ic0:ic0 + wc, :]
                    nc.tensor.matmul(
                        out, lhsT, rhs, start=(cnt == 0),
                        stop=(finish and cnt == TOTAL_BK[bk] - 1),
                        perf_mode=DR)
                    if cnt + 1 == TOTAL_BK[bk] and on_bank_done:
                        on_bank_done(ii, oh, bk, ps[ii][oh][bk])
        if (interleave is not None and j >= interleave_skip
                and (j - interleave_skip) % interleave_every
                == interleave_every - 1):
            interleave()
    return ps


def build_kernel():
    """Build the full per-core kernel (same NEFF on all 8 cores)."""
    nc = bacc.Bacc("TRN2", target_bir_lowering=False, debug=False,
                   num_devices=NCORES)
    dt = {}
    for nmm in ("q", "k", "v"):
        dt[f"x{nmm}"] = nc.dram_tensor(f"x{nmm}", [128, 2, 8, 8, BL], FP8,
                                       kind="ExternalInput")
        dt[f"w{nmm}"] = nc.dram_tensor(f"w{nmm}", [128, NTAPS, 2, 256], FP8,
                                       kind="ExternalInput")
        dt[f"bias{nmm}"] = nc.dram_tensor(f"bias{nmm}", [2, 128], F32,
                                          kind="ExternalInput")
    dt["bias2v"] = nc.dram_tensor("bias2v", [2, 128], F32,
                                  kind="ExternalInput")
    dt["wo_t"] = nc.dram_tensor("wo_t", [2, 128, 256], BF16,
                                kind="ExternalInput")
    dt["bo"] = nc.dram_tensor("bo", [1, 256], BF16, kind="ExternalInput")
    dt["bias2k"] = nc.dram_tensor("bias2k", [1, 256], BF16,
                                  kind="ExternalInput")
    dt["ones"] = nc.dram_tensor("ones", [1, 128], BF16, kind="ExternalInput")
    dt["ident"] = nc.dram_tensor("ident", [128, 128], BF16,
                                 kind="ExternalInput")
    dt["ident64"] = nc.dram_tensor("ident64", [128, 64], BF16,
                                   kind="ExternalInput")
    dt["out"] = nc.dram_tensor("out", [8, 128, 256], F32,
                               kind="ExternalOutput")

    with TileContext(nc) as tc:
      with tc.tile_pool(name="persist", bufs=1) as pp:
        # --- persistent SBUF ---
        wv_t = pp.tile([128, NTAPS, 2, 256], FP8, name="wv_t")
        wk_t = pp.tile([128, NTAPS, 2, 256], FP8, name="wk_t")
        bias_t = {}
        for nmm in ("q", "k", "v"):
            bias_t[nmm] = pp.tile([128, 2], F32, name=f"bias{nmm}_t")
        bias2v_t = pp.tile([128, 2], F32, name="bias2v_t")
        ones_t = pp.tile([1, 128], BF16, name="ones_t")
        ones512_t = pp.tile([1, 512], BF16, name="ones512_t")
        biasw_t = pp.tile([1, 256], BF16, name="biasw_t")
        bo_t = pp.tile([1, 256], BF16, name="bo_t")
        ident_t = pp.tile([128, 128], BF16, name="ident_t")
        ident64_t = pp.tile([128, 64], BF16, name="ident64_t")
        ones32_t = pp.tile([128, 32], BF16, name="ones32_t")
        wo_tt = [pp.tile([128, 256], BF16, name=f"wo_tt{h}") for h in range(2)]
        # conv2 outputs [c, b, pix] — live into attention
        hh = {}
        for nmm in ("q", "k", "v"):
            hh[nmm] = [pp.tile([128, BL, NPIX], BF16, name=f"h{nmm}{h}")
                       for h in range(2)]
        E_t = pp.tile([128, BL // 2, H, NPIX], BF16, name="E_t")
        VT = pp.tile([128, BL // 2, H, 33], BF16, name="VT")
        rcp = pp.tile([128, BL // 2, H], F32, name="rcp")
        OAu = pp.tile([128, BL // 2, H, 33], BF16, name="OAu")
        OA = pp.tile([128, BL // 2, 2, 128], BF16, name="OA")
        concat = [pp.tile([128, BL, NPIX], BF16, name=f"concat{h}")
                  for h in range(2)]

        def load_x(pool, nmm, eng):
            xt = pool.tile([128, 2, 8, 8, BL], FP8, name=f"x{nmm}t")
            eng.dma_start(xt[:], dt[f"x{nmm}"].ap())
            return xt

        def relu_hook(x1t, bias):
            # psum [o,4,8,b] -> x1 fp8 quadrant,
            # out = relu(psum*S1 + bias*AS) = 2^6 * relu(y1 + b)
            def f(ii, oh, bk, pst):
                nc.scalar.activation(
                    x1t[:, oh, bk * 4:(bk + 1) * 4, :, :],
                    pst[:],
                    mybir.ActivationFunctionType.Relu,
                    bias=bias[:, oh:oh + 1], scale=S1)
            return f

        def vh_hook(out_t, bias):
            # v-branch conv2 drain: divide out F2, add bv -> vh bf16
            def f(ii, oh, bk, pst):
                nc.scalar.activation(
                    out_t[oh][:, :, bk * 32:(bk + 1) * 32]
                    .rearrange("c b (r q) -> c b r q", r=4),
                    pst[:].rearrange("c r q b -> c b r q"),
                    mybir.ActivationFunctionType.Identity,
                    bias=bias[:, oh:oh + 1], scale=1.0 / F2)
            return f

        def kq_hook(ii, oh, bk, pst):
            # conv2 k/q drain: bias into psum (bk*F2 outer product),
            # then DVE copies (first 2 batches first so the attention's
            # scores unblock early).
            nm = ('k', 'q')[ii]
            nc.tensor.matmul(
                pst[:].rearrange("c r q b -> c (r q b)"),
                biasw_t[:, oh * 128:(oh + 1) * 128],
                ones512_t[:], start=False, stop=True)
            for b0, b1 in ((0, 2), (2, 16)):
                nc.vector.tensor_copy(
                    hh[nm][oh][:, b0:b1, bk * 32:(bk + 1) * 32]
                    .rearrange("c b (r q) -> c b r q", r=4),
                    pst[:, :, :, b0:b1]
                    .rearrange("c r q b -> c b r q"))

        # ================= attention emitters =================
        # Heads hp 0-3 of c-half `oh` live in kh/qh[oh]; a head's whole
        # chain needs only that half.  Psum comes from the conv pool's
        # tag rings: `ptags` names 2 tags for scores and 2 for AV/OT.
        kh, qh = hh['k'], hh['q']

        def emit_scores_h(ps_pool, ptags, b2, oh, salt):
            for hp in range(4):
                h = oh * 4 + hp
                pst = ps_pool.tile([128, NPIX], F32,
                                   tag=ptags[hp % len(ptags)],
                                   name=f"pst{salt}{b2}{h}",
                                   padded_shape=[128, 512])
                for par in range(2):
                    b = 2 * b2 + par
                    nc.tensor.matmul(
                        pst[64 * par:64 * par + 64, :],
                        kh[oh][32 * hp:32 * hp + 32, b, :],
                        qh[oh][32 * hp:32 * hp + 32, b, :],
                        start=True, stop=True,
                        tile_position=(32 * hp, 64 * par))
                nc.scalar.activation(
                    E_t[:, b2, h, :], pst[:],
                    mybir.ActivationFunctionType.Exp, scale=EXPS)

        def emit_av_h(ps_pool, ptags, b2, oh, salt):
            for hp in range(4):
                h = oh * 4 + hp
                pso = ps_pool.tile([128, 33], F32,
                                   tag=ptags[hp % len(ptags)],
                                   name=f"pso{salt}{b2}{h}",
                                   padded_shape=[128, 512])
                for par in range(2):
                    nc.tensor.matmul(
                        pso[64 * par:64 * par + 64, :],
                        E_t[64 * par:64 * par + 64, b2, h, :],
                        VT[64 * par:64 * par + 64, b2, h, :],
                        start=True, stop=True,
                        tile_position=(64 * par, 64 * par))
                nc.vector.tensor_copy(OAu[:, b2, h, :], pso[:])
            nc.vector.reciprocal(rcp[:, b2, oh * 4:oh * 4 + 4],
                                 OAu[:, b2, oh * 4:oh * 4 + 4, 32:33])
            for hp in range(4):
                h = oh * 4 + hp
                nc.vector.tensor_scalar_mul(
                    OA[:, b2, oh, 32 * hp:32 * hp + 32],
                    OAu[:, b2, h, 0:32], rcp[:, b2, h:h + 1])

        def emit_ot(ps_pool, ptag, b2, oh2, salt):
            for par in range(2):
                b = 2 * b2 + par
                pot = ps_pool.tile([128, 64], BF16, tag=ptag,
                                   name=f"pot{salt}{b}{oh2}",
                                   padded_shape=[128, 1024])
                nc.tensor.transpose(
                    pot[:], OA[64 * par:64 * par + 64, b2, oh2, :],
                    ident64_t[64 * par:64 * par + 64, :],
                    tile_position=(64 * par, 0))
                nc.vector.tensor_copy(concat[oh2][:, b, :], pot[:])

        with tc.tile_pool(name="cdata", bufs=1) as cd, \
             tc.tile_pool(name="convps", bufs=1, space="PSUM") as cvp, \
             tc.tile_pool(name="attnsb", bufs=1) as asb:
            # ---- DMA emission in deadline order.  xv rides the scalar
            # queue in parallel with wv's serial sync-queue chunks so
            # P4's first matmuls unblock as early as possible. ----
            xv = load_x(cd, 'v', nc.scalar)
            wv_ch = load_weights_resident(nc, nc.sync, wv_t, dt["wv"].ap(),
                                          chunk_plan(2, 4), par_head=6)
            nc.vector.memset(VT[:, :, :, 32:33], 1.0)
            nc.vector.memset(ones32_t[:], 1.0)
            # ---- PE warm-up: the HAM clock-gate needs ~3.4us of
            # sustained activity to lift the PE from 1.2 to 2.4 GHz.
            # The input DMAs take ~9us to land; burn that dead time on
            # throwaway matmuls so the first real conv runs warm. ----
            scratch = cd.tile([128, 512], BF16, name="scratch")
            nc.vector.memset(scratch[:], 1.0)
            warm_ps = cvp.tile([128, 512], F32, tag="cvB01",
                               name="warm_ps")
            for wi in range(22):
                nc.tensor.matmul(warm_ps[:], scratch[:, 0:128],
                                 scratch[:], start=True, stop=True)
            xq = load_x(cd, 'q', nc.gpsimd)
            xk = load_x(cd, 'k', nc.gpsimd)
            wk_ch = load_weights_resident(nc, nc.sync, wk_t, dt["wk"].ap(),
                                          chunk_plan(8, 8))
            for nmm in ("q", "k", "v"):
                nc.gpsimd.dma_start(bias_t[nmm][:],
                                    dt[f"bias{nmm}"].ap()
                                    .rearrange("h c -> c h"))
            nc.gpsimd.dma_start(bias2v_t[:],
                                dt["bias2v"].ap().rearrange("h c -> c h"))
            nc.gpsimd.dma_start(ones_t[:], dt["ones"][:])
            nc.vector.memset(ones512_t[:], 1.0)
            nc.gpsimd.dma_start(biasw_t[:], dt["bias2k"][:])
            nc.gpsimd.dma_start(bo_t[:], dt["bo"][:])
            nc.scalar.dma_start(ident_t[:], dt["ident"][:])
            nc.scalar.dma_start(ident64_t[:], dt["ident64"][:])
            for h in range(2):
                nc.scalar.dma_start(wo_tt[h][:], dt["wo_t"][h])
            x1 = {}
            for nmm in ("q", "k", "v"):
                x1[nmm] = cd.tile([128, 2, 8, 8, BL], FP8, name=f"x1{nmm}t")

            # ---- P4: v -> v1 (desc order = storage/DMA order) ----
            conv_pass(nc, cvp, wv_ch, [xv], tag="p4", ptags=["cvA"],
                      on_bank_done=relu_hook(x1['v'], bias_t['v']))
            # ---- P5: v1 -> vh (asc order: reads rows 0-3 first) ----
            conv_pass(nc, cvp, wv_ch, [x1['v']], tag="p5", ptags=["cvB"],
                      order=ASC, on_bank_done=vh_hook(hh['v'], bias2v_t))

            # ---- V transposes: vh [c,b,pix] -> VT [kpix, b2, h, dk|1] ----
            # interleaved into P1's matmul stream to keep HAM warm;
            # skipped for the first taps so P5's drain ACTs clear first
            vt_jobs = []
            for b in range(BL):
                par, b2 = b % 2, b // 2
                for oh in range(2):
                    vt_jobs.append((b, par, b2, oh))
            vt_state = {'i': 0}

            def emit_vt(njobs=6):
                for _ in range(njobs):
                    i = vt_state['i']
                    if i >= len(vt_jobs):
                        return
                    vt_state['i'] = i + 1
                    b, par, b2, oh = vt_jobs[i]
                    pvt = cvp.tile([64, 128], BF16, tag=f"cvB0{i % 2}",
                                   name=f"pvt{b}{oh}")
                    nc.tensor.transpose(pvt[:], hh['v'][oh][:, b, :],
                                        ident_t[:])
                    nc.vector.tensor_copy(
                        VT[64 * par:64 * par + 64, b2,
                           oh * 4:(oh + 1) * 4, 0:32],
                        pvt[:].rearrange("k (h d) -> k h d", h=4))

            # ---- P1: q -> q1 (stream wq, storage order) ----
            with tc.tile_pool(name="wstream", bufs=3) as wsp:
                wq_ch = load_weights_stream(nc, wsp, dt["wq"].ap(), tag="q")
                conv_pass(nc, cvp, wq_ch, [xq], tag="p1", ptags=["cvA"],
                          interleave=emit_vt, interleave_skip=8,
                          on_bank_done=relu_hook(x1['q'], bias_t['q']))
                emit_vt(len(vt_jobs))    # any leftovers

            # ---- P2: k -> k1 (desc) ----
            conv_pass(nc, cvp, wk_ch, [xk], tag="p2", ptags=["cvA"],
                      on_bank_done=relu_hook(x1['k'], bias_t['k']))

            # ---- P3a: {k1, q1} -> kh/qh oh=0 half (asc, shared LS) ----
            conv_pass(nc, cvp, wk_ch, [x1['k'], x1['q']], tag="p3a",
                      ptags=["cvA", "cvB"], order=ASC, oh_list=(0,),
                      finish=False, on_bank_done=kq_hook)

            # ---- P3b: oh=1 half, with heads 0-3's attention chain
            # interleaved between its conv matmuls (they only need the
            # oh=0 outputs + VT).  Psum rides the freed oh=0 tag rings.
            attn_jobs = []
            for s in range(BL // 2 + 3):
                if s < BL // 2:
                    attn_jobs.append(('sc', s))
                if 2 <= s < BL // 2 + 2:
                    attn_jobs.append(('av', s - 2))
                if 3 <= s:
                    attn_jobs.append(('ot', s - 3))
            aj_state = {'i': 0}

            def emit_attn_a(njobs=1):
                for _ in range(njobs):
                    i = aj_state['i']
                    if i >= len(attn_jobs):
                        return
                    aj_state['i'] = i + 1
                    op, b2 = attn_jobs[i]
                    if op == 'sc':
                        emit_scores_h(cvp, ["cvA00", "cvA01"], b2, 0, 'a')
                    elif op == 'av':
                        emit_av_h(cvp, ["cvB00", "cvB01"], b2, 0, 'a')
                    else:
                        emit_ot(cvp, ["cvA00", "cvA01"][b2 % 2], b2, 0, 'a')

            conv_pass(nc, cvp, wk_ch, [x1['k'], x1['q']], tag="p3b",
                      ptags=["cvA", "cvB"], order=ASC, oh_list=(1,),
                      finish=False, on_bank_done=kq_hook,
                      interleave=emit_attn_a, interleave_skip=4,
                      interleave_every=2)
            emit_attn_a(len(attn_jobs))    # flush leftovers

            # ---- attention tail: heads 4-7 + projection, pipelined
            # per batch-pair: sc(b2) | av(b2-1) | proj(b2-2).  AV here
            # runs V-stationary: out[dk, qpix] lands [c, pix]-shaped (4
            # heads packed via col groups), a ones-stationary matmul
            # replicates each head's rowsum across its dk partitions,
            # and one reciprocal + one elementwise multiply writes
            # concat directly — no O-transposes.
            def emit_av_b(b2):
                for par in range(2):
                    av = cvp.tile([128, NPIX], F32,
                                  tag=["cvA10", "cvA11"][par],
                                  name=f"avb{b2}{par}",
                                  padded_shape=[128, 512])
                    rs = cvp.tile([128, NPIX], F32,
                                  tag=["cvB10", "cvB11"][par],
                                  name=f"rsb{b2}{par}",
                                  padded_shape=[128, 512])
                    for hp in range(4):
                        h = 4 + hp
                        nc.tensor.matmul(
                            av[32 * hp:32 * hp + 32, :],
                            VT[64 * par:64 * par + 64, b2, h, 0:32],
                            E_t[64 * par:64 * par + 64, b2, h, :],
                            start=True, stop=True,
                            tile_position=(64 * par, 32 * hp))
                        nc.tensor.matmul(
                            rs[32 * hp:32 * hp + 32, :],
                            ones32_t[64 * par:64 * par + 64, :],
                            E_t[64 * par:64 * par + 64, b2, h, :],
                            start=True, stop=True,
                            tile_position=(64 * par, 32 * hp))
                    rr = asb.tile([128, NPIX], F32, tag="rcpB",
                                  name=f"rr{b2}{par}", bufs=2)
                    nc.vector.reciprocal(rr[:], rs[:])
                    nc.vector.tensor_mul(concat[1][:, 2 * b2 + par, :],
                                         av[:], rr[:])

            def emit_proj(blk):
                pspr = cvp.tile([128, 256], F32, tag="cvB00",
                                name=f"pspr{blk}",
                                padded_shape=[128, 512])
                for oh in range(2):
                    nc.tensor.matmul(
                        pspr[:],
                        concat[oh].rearrange("c b p -> c (b p)")
                        [:, blk * 128:(blk + 1) * 128],
                        wo_tt[oh][:], start=(oh == 0), stop=False)
                nc.tensor.matmul(pspr[:], ones_t[:], bo_t[:],
                                 start=False, stop=True)
                osb = asb.tile([128, 256], F32, tag="osb",
                               name=f"osb{blk}", bufs=2)
                nc.vector.tensor_copy(osb[:], pspr[:])
                nc.sync.dma_start(dt["out"][blk], osb[:])

            for step in range(BL // 2 + 2):
                if step < BL // 2:
                    emit_scores_h(cvp, ["cvA00", "cvA01"], step, 1, 'b')
                if 1 <= step < BL // 2 + 1:
                    emit_av_b(step - 1)
                if 2 <= step:
                    emit_proj(step - 2)
    nc.compile()
    return nc


# ---------------------------------------------------------------------------
# Host-side prep
# ---------------------------------------------------------------------------

def prep_weights(w):
    """w: [D, D, 15, 15] OIHW -> [128, NTAPS, 2, 256] fp8e4 laid out
    (c_lo, tap, c-half, o), scaled by WS."""
    wt = np.empty((NTAPS, 2, 128, 256), np.float32)
    for i, (sr, sc, *_r) in enumerate(TAPS):
        # [O, I] -> [I, O] -> [ch, c_lo, O]
        wt[i] = (w[:, :, sr + 7, sc + 7].T * WS).reshape(2, 128, 256)
    wt = wt.transpose(2, 0, 1, 3)   # -> [c_lo, tap, ch, o]
    return np.ascontiguousarray(wt).astype(ml_dtypes.float8_e4m3)


def prep_static(wk, bk, wq, bq, wv, bv, wo, bo):
    """Host-side weight prep shared by all cores."""
    st = {}
    for nmm, w, b in (("q", wq, bq), ("k", wk, bk), ("v", wv, bv)):
        st[f"w{nmm}"] = prep_weights(np.asarray(w, np.float32))
        st[f"bias{nmm}"] = np.ascontiguousarray(
            (np.asarray(b, np.float32) * AS).reshape(2, 128))
    st["bias2v"] = np.ascontiguousarray(
        np.asarray(bv, np.float32).reshape(2, 128))
    st["wo_t"] = np.ascontiguousarray(
        np.asarray(wo, np.float32).T).reshape(2, 128, 256).astype(
        ml_dtypes.bfloat16)
    st["bo"] = np.asarray(bo, np.float32).reshape(1, 256).astype(
        ml_dtypes.bfloat16)
    st["ones"] = np.ones((1, 128), ml_dtypes.bfloat16)
    st["bias2k"] = (np.asarray(bk, np.float32) * F2).reshape(1, 256).astype(
        ml_dtypes.bfloat16)
    st["ident"] = np.eye(128, dtype=ml_dtypes.bfloat16)
    st["ident64"] = np.tile(np.eye(64, dtype=ml_dtypes.bfloat16), (2, 1))
    return st


def prep_core_x(x, core):
    """x: [B, 8, 8, D] -> this core's [128, 2, 8, 8, BL] fp8
    (c_lo, c-half, row, col, b), scaled by XS."""
    xs = np.asarray(x[core * BL:(core + 1) * BL], np.float32) * XS
    xs = xs.transpose(3, 1, 2, 0)                    # [D, r, c, b]
    xs = xs.reshape(2, 128, 8, 8, BL).transpose(1, 0, 2, 3, 4)
    return np.ascontiguousarray(np.clip(xs, -240, 240)).astype(
        ml_dtypes.float8_e4m3)


def make_in_maps(q, k, v, st):
    in_maps = []
    for core in range(NCORES):
        m = dict(st)
        m["xq"] = prep_core_x(q, core)
        m["xk"] = prep_core_x(k, core)
        m["xv"] = prep_core_x(v, core)
        in_maps.append(m)
    return in_maps


def gather_out(results):
    """results: list of dicts with 'out' [8, 128, 256] -> [B, 8, 8, D]."""
    outs = [r["out"].reshape(BL, 8, 8, D) for r in results]
    return np.concatenate(outs, axis=0)


# ---------------------------------------------------------------------------
# Self-contained entry point: kernel(**inputs) -> full [128, 8, 8, 256]
# ---------------------------------------------------------------------------
_NC_CACHE = None


def _get_nc():
    global _NC_CACHE
    if _NC_CACHE is None:
        _NC_CACHE = build_kernel()
    return _NC_CACHE


def kernel(q, k, v, wk, bk, wq, bq, wv, bv, wo, bo):
    nc = _get_nc()
    st = prep_static(wk, bk, wq, bq, wv, bv, wo, bo)
    in_maps = make_in_maps(np.asarray(q), np.asarray(k), np.asarray(v), st)
    res = bass_utils.run_bass_kernel_spmd(
        nc, in_maps, core_ids=list(range(NCORES)))
    return gather_out(res.results)


# revision 19
# speedup vs baseline: 1.0400x; 1.0317x over previous
"""MultiHeadDoubleAttention TRN2 kernel — v5 fp8-DoubleRow convs,
attention half-hidden under the last conv subpass.

Data-parallel over batch: 8 cores x 16 batch each.

Conv: 15x15 hollow-masked conv on an 8x8 grid == 65 shift-taps of
channel matmuls.  Conv matmuls run fp8e4 (TRN E4M3, max 240) in
DoubleRow perf mode: K=256 contraction per instruction (both c-halves
in one pass), 2 multiplies/cell/cycle -> 2x bf16 streaming rate.
Correctness gate is 2e-2; fp8 convs measure ~6e-3 end to end.

Layouts (all per-core):
  activations  [c=128, ch=2, row=8, col=8, b=16] fp8  -- a tap piece's
    rhs is [p, 2, ph, wc*16]: (col,b) merge into one contiguous AP dim.
  weights      [c=128, tap=65, ch=2, o=256] fp8, lhsT per (tap, oh) =
    [p, 2, 128].
  conv psum    [o=128, r=4, c=8, b=16] f32 = one full bank per
    (oh, row-half).

Fixed power-of-2 scales keep fp8 in range: w*2^15, inputs*2^4,
conv1-out*2^6 (folded into the relu-drain ACT).  Conv2 psum carries
2^21; the v-branch drain divides it out (vh unscaled), k/q keep it in
bf16 and the attention exp folds 2^-42 into its scale.

Pipelining:
  - Taps are stored (and DMA'd) in sr-DESCENDING order; passes that
    consume a previous pass's output iterate sr-ASCENDING, so each
    pass's first matmuls depend only on quadrants its producer drained
    mid-stream (per-(oh, row-half) bank hooks fire drains the moment a
    bank's accumulation completes).
  - k/q conv2 runs as two 2-input subpasses (one per output c-half),
    sharing each LDWEIGHTS between the k and q streams.  Heads 0-3 of
    the attention need only the oh=0 half of kh/qh — their full
    scores/exp/AV/normalize/O-transpose chain is interleaved between
    the oh=1 subpass's conv matmuls (PE stays dense and the HAM clock
    never cools); only heads 4-7 + projection remain as a tail.
  - Attention psum lives in the conv pool's tag rings (PSUM is exactly
    8 banks; tags are reused once their conv phase drains).
"""
import sys
sys.path.insert(0, '/opt/trn_rl_repo')
import numpy as np
import ml_dtypes

import concourse.bass as bass
import concourse.bacc as bacc
import concourse.mybir as mybir
import concourse.bass_utils as bass_utils
from concourse.tile import TileContext

F32 = mybir.dt.float32
BF16 = mybir.dt.bfloat16
FP8 = mybir.dt.float8e4
DR = mybir.MatmulPerfMode.DoubleRow

B, D, H, DK = 128, 256, 8, 32
NCORES = 8
BL = B // NCORES          # batch per core
NPIX = 64                 # 8x8
RS = 1.0 / np.sqrt(DK)    # score scale

WS = float(2 ** 15)       # conv weight scale
XS = float(2 ** 4)        # input activation scale
AS = float(2 ** 6)        # conv1-output activation scale
S1 = AS / (WS * XS)       # conv1 psum -> x1 drain scale (2^-13)
F2 = WS * AS              # scale carried by conv2 psum (2^21)
EXPS = RS / (F2 * F2)     # exp scale absorbing kh/qh carry


def hollow_mask():
    m = np.ones((15, 15), np.float32)
    for c in range(5):
        m[1 + c:7, c] = 0; m[8:14 - c, c] = 0
        m[c, 1 + c:7] = 0; m[c, 8:14 - c] = 0
        m[1 + c:7, 14 - c] = 0; m[8:14 - c, 14 - c] = 0
        m[14 - c, 1 + c:7] = 0; m[14 - c, 8:14 - c] = 0
    return m


def tap_schedule():
    """All 65 unmasked taps as (sr, sc, ar0, hr, ac0, wc), stored in
    sr-DESCENDING order (bank0 finishes early when iterated in storage
    order; iterate ASC for the reverse)."""
    m = hollow_mask()
    taps = []
    for di in range(15):
        for dj in range(15):
            if not m[di, dj]:
                continue
            sr, sc = di - 7, dj - 7
            ar0, ar1 = max(0, sr), min(7, 7 + sr)
            ac0, ac1 = max(0, sc), min(7, 7 + sc)
            taps.append((sr, sc, ar0, ar1 - ar0 + 1, ac0, ac1 - ac0 + 1))
    taps.sort(key=lambda e: (-e[0], -(e[3] * e[5])))
    return taps


def tap_pieces(sr, sc, ar0, hr, ac0, wc):
    """Split a tap's output rect at the ir=4 psum-bank boundary.
    Returns list of (bank, ir0_in_bank, ar0, ph, ic0, ac0, wc)."""
    ir0 = ar0 - sr
    ic0 = ac0 - sc
    pieces = []
    lo, hi = ir0, ir0 + hr
    if lo < 4:
        ph = min(hi, 4) - lo
        pieces.append((0, lo, lo + sr, ph, ic0, ac0, wc))
    if hi > 4:
        p0 = max(lo, 4)
        ph = hi - p0
        pieces.append((1, p0 - 4, p0 + sr, ph, ic0, ac0, wc))
    return pieces


TAPS = tap_schedule()
NTAPS = len(TAPS)                       # 65
PIECES = [tap_pieces(*t) for t in TAPS]
# ascending-sr iteration order (indices into storage order):
ASC = sorted(range(NTAPS), key=lambda i: (TAPS[i][0], -TAPS[i][3] * TAPS[i][5]))
# accumulation counts per psum bank half (same for every input / oh):
TOTAL_BK = {0: 0, 1: 0}
for _pl in PIECES:
    for _p in _pl:
        TOTAL_BK[_p[0]] += 1


def chunk_plan(first, rest):
    """Tap chunk sizes [first, rest, rest, ...] covering NTAPS."""
    plan = []
    c0 = 0
    while c0 < NTAPS:
        n = min(first if c0 == 0 else rest, NTAPS - c0)
        plan.append((c0, n))
        c0 += n
    return plan


def load_weights_resident(nc, eng, tile, w_dram, plan, par_head=0):
    """Chunked DMAs of a full conv weight tensor into one resident tile.
    The first `par_head` chunk triggers round-robin over the three
    DMA-capable queues so their transfers land in parallel (per-chunk
    tile deps make arrival order irrelevant); the rest ride `eng`.
    Returns [(c0, n, tile_slice)] for conv_pass."""
    chunks = []
    head_engines = [nc.sync, nc.scalar, nc.gpsimd]
    for i, (c0, n) in enumerate(plan):
        e = head_engines[i % 3] if i < par_head else eng
        e.dma_start(tile[:, c0:c0 + n], w_dram[:, c0:c0 + n])
        chunks.append((c0, n, tile[:, c0:c0 + n]))
    return chunks


def load_weights_stream(nc, pool, w_dram, tag, chunk=4):
    """Rotating-chunk DMAs for a use-once conv weight tensor."""
    chunks = []
    for c0, n in chunk_plan(chunk, chunk):
        wt = pool.tile([128, chunk, 2, 256], FP8, tag=f"{tag}wS",
                       name=f"{tag}wS{c0}")
        nc.sync.dma_start(wt[:, :n], w_dram[:, c0:c0 + n])
        chunks.append((c0, n, wt[:, :n]))
    return chunks


def conv_pass(nc, psum_pool, chunks, inputs, tag, ptags, order=None,
              oh_list=(0, 1), interleave=None, interleave_skip=0,
              interleave_every=4, finish=True, on_bank_done=None):
    """One DoubleRow conv layer over `inputs` (list of fp8 tiles
    [128, 2, 8, 8, BL]) for the output-channel halves in `oh_list`.
    Accumulates into psum tiles [128, 4, 8, BL] per (input, o-half,
    pixel-row-half).  `order` permutes tap iteration (storage order if
    None).  `on_bank_done(ii, oh, bk, ps)` fires right after the
    matmul that completes a bank."""
    ps = [{oh: [psum_pool.tile([128, 4, 8, BL], F32,
                               tag=f"{ptags[ii]}{oh}{bk}",
                               name=f"{tag}ps{ii}{oh}{bk}")
               for bk in range(2)] for oh in oh_list}
          for ii in range(len(inputs))]
    items = []
    for c0, n, wt in chunks:
        for tl in range(n):
            items.append((c0 + tl, wt, tl))
    if order is not None:
        items = [items[i] for i in order]
    done = {}
    for j, (ti, wt, tl) in enumerate(items):
        pieces = PIECES[ti]
        for oh in oh_list:
            lhsT = wt[:, tl, :, oh * 128:(oh + 1) * 128]
            for ii, x in enumerate(inputs):
                for (bk, irb, ar0, ph, ic0, ac0, wc) in pieces:
                    key = (ii, oh, bk)
                    cnt = done.get(key, 0)
                    done[key] = cnt + 1
                    rhs = x[:, :, ar0:ar0 + ph, ac0:ac0 + wc, :]
                    out = ps[ii][oh][bk][:, irb:irb + ph,
                                         ic0:ic0 + wc, :]
                    nc.tensor.matmul(
                        out, lhsT, rhs, start=(cnt == 0),
                        stop=(finish and cnt == TOTAL_BK[bk] - 1),
                        perf_mode=DR)
                    if cnt + 1 == TOTAL_BK[bk] and on_bank_done:
                        on_bank_done(ii, oh, bk, ps[ii][oh][bk])
        if (interleave is not None and j >= interleave_skip
                and (j - interleave_skip) % interleave_every
                == interleave_every - 1):
            interleave()
    return ps


def build_kernel():
    """Build the full per-core kernel (same NEFF on all 8 cores)."""
    nc = bacc.Bacc("TRN2", target_bir_lowering=False, debug=False,
                   num_devices=NCORES)
    dt = {}
    for nmm in ("q", "k", "v"):
        dt[f"x{nmm}"] = nc.dram_tensor(f"x{nmm}", [128, 2, 8, 8, BL], FP8,
                                       kind="ExternalInput")
        dt[f"w{nmm}"] = nc.dram_tensor(f"w{nmm}", [128, NTAPS, 2, 256], FP8,
                                       kind="ExternalInput")
        dt[f"bias{nmm}"] = nc.dram_tensor(f"bias{nmm}", [2, 128], F32,
                                          kind="ExternalInput")
    dt["bias2v"] = nc.dram_tensor("bias2v", [2, 128], F32,
                                  kind="ExternalInput")
    dt["wo_t"] = nc.dram_tensor("wo_t", [2, 128, 256], BF16,
                                kind="ExternalInput")
    dt["bo"] = nc.dram_tensor("bo", [1, 256], BF16, kind="ExternalInput")
    dt["bias2k"] = nc.dram_tensor("bias2k", [1, 256], BF16,
                                  kind="ExternalInput")
    dt["ones"] = nc.dram_tensor("ones", [1, 128], BF16, kind="ExternalInput")
    dt["ident"] = nc.dram_tensor("ident", [128, 128], BF16,
                                 kind="ExternalInput")
    dt["out"] = nc.dram_tensor("out", [8, 128, 256], F32,
                               kind="ExternalOutput")

    with TileContext(nc) as tc:
      with tc.tile_pool(name="persist", bufs=1) as pp:
        # --- persistent SBUF ---
        wv_t = pp.tile([128, NTAPS, 2, 256], FP8, name="wv_t")
        wk_t = pp.tile([128, NTAPS, 2, 256], FP8, name="wk_t")
        bias_t = {}
        for nmm in ("q", "k", "v"):
            bias_t[nmm] = pp.tile([128, 2], F32, name=f"bias{nmm}_t")
        bias2v_t = pp.tile([128, 2], F32, name="bias2v_t")
        ones_t = pp.tile([1, 128], BF16, name="ones_t")
        ones512_t = pp.tile([1, 512], BF16, name="ones512_t")
        biasw_t = pp.tile([1, 256], BF16, name="biasw_t")
        bo_t = pp.tile([1, 256], BF16, name="bo_t")
        ident_t = pp.tile([128, 128], BF16, name="ident_t")
        ones32_t = pp.tile([128, 32], BF16, name="ones32_t")
        wo_tt = [pp.tile([128, 256], BF16, name=f"wo_tt{h}") for h in range(2)]
        # conv2 outputs [c, b, pix] — live into attention
        hh = {}
        for nmm in ("q", "k", "v"):
            hh[nmm] = [pp.tile([128, BL, NPIX], BF16, name=f"h{nmm}{h}")
                       for h in range(2)]
        E_t = pp.tile([128, BL // 2, H, NPIX], BF16, name="E_t")
        VT = pp.tile([128, BL // 2, H, 33], BF16, name="VT")
        concat = [pp.tile([128, BL, NPIX], BF16, name=f"concat{h}")
                  for h in range(2)]

        def load_x(pool, nmm, eng):
            xt = pool.tile([128, 2, 8, 8, BL], FP8, name=f"x{nmm}t")
            eng.dma_start(xt[:], dt[f"x{nmm}"].ap())
            return xt

        def relu_hook(x1t, bias):
            # psum [o,4,8,b] -> x1 fp8 quadrant,
            # out = relu(psum*S1 + bias*AS) = 2^6 * relu(y1 + b)
            def f(ii, oh, bk, pst):
                nc.scalar.activation(
                    x1t[:, oh, bk * 4:(bk + 1) * 4, :, :],
                    pst[:],
                    mybir.ActivationFunctionType.Relu,
                    bias=bias[:, oh:oh + 1], scale=S1)
            return f

        def vh_hook(out_t, bias):
            # v-branch conv2 drain: divide out F2, add bv -> vh bf16
            def f(ii, oh, bk, pst):
                nc.scalar.activation(
                    out_t[oh][:, :, bk * 32:(bk + 1) * 32]
                    .rearrange("c b (r q) -> c b r q", r=4),
                    pst[:].rearrange("c r q b -> c b r q"),
                    mybir.ActivationFunctionType.Identity,
                    bias=bias[:, oh:oh + 1], scale=1.0 / F2)
            return f

        def kq_hook(ii, oh, bk, pst):
            # conv2 k/q drain: bias into psum (bk*F2 outer product),
            # then DVE copies (first 2 batches first so the attention's
            # scores unblock early).
            nm = ('k', 'q')[ii]
            nc.tensor.matmul(
                pst[:].rearrange("c r q b -> c (r q b)"),
                biasw_t[:, oh * 128:(oh + 1) * 128],
                ones512_t[:], start=False, stop=True)
            for b0, b1 in ((0, 2), (2, 16)):
                nc.vector.tensor_copy(
                    hh[nm][oh][:, b0:b1, bk * 32:(bk + 1) * 32]
                    .rearrange("c b (r q) -> c b r q", r=4),
                    pst[:, :, :, b0:b1]
                    .rearrange("c r q b -> c b r q"))

        # ================= attention emitters =================
        # Heads hp 0-3 of c-half `oh` live in kh/qh[oh]; a head's whole
        # chain needs only that half.  Psum comes from the conv pool's
        # tag rings: `ptags` names 2 tags for scores and 2 for AV/OT.
        kh, qh = hh['k'], hh['q']

        def emit_scores_h(ps_pool, ptags, b2, oh, salt):
            for hp in range(4):
                h = oh * 4 + hp
                pst = ps_pool.tile([128, NPIX], F32,
                                   tag=ptags[hp % len(ptags)],
                                   name=f"pst{salt}{b2}{h}",
                                   padded_shape=[128, 512])
                for par in range(2):
                    b = 2 * b2 + par
                    nc.tensor.matmul(
                        pst[64 * par:64 * par + 64, :],
                        kh[oh][32 * hp:32 * hp + 32, b, :],
                        qh[oh][32 * hp:32 * hp + 32, b, :],
                        start=True, stop=True,
                        tile_position=(32 * hp, 64 * par))
                nc.scalar.activation(
                    E_t[:, b2, h, :], pst[:],
                    mybir.ActivationFunctionType.Exp, scale=EXPS)

        def emit_av_sw(b2, oh, avt, rst, salt):
            # V-stationary AV: out[dk, qpix] lands [c, pix]-shaped (4
            # heads packed via col groups); a ones-stationary matmul
            # replicates each head's rowsum across its dk partitions;
            # one reciprocal + one elementwise multiply writes concat
            # directly — no O-transposes.
            for par in range(2):
                av = cvp.tile([128, NPIX], F32, tag=avt[par % len(avt)],
                              name=f"av{salt}{b2}{par}",
                              padded_shape=[128, 512])
                rs = cvp.tile([128, NPIX], F32, tag=rst[par % len(rst)],
                              name=f"rs{salt}{b2}{par}",
                              padded_shape=[128, 512])
                for hp in range(4):
                    h = oh * 4 + hp
                    nc.tensor.matmul(
                        av[32 * hp:32 * hp + 32, :],
                        VT[64 * par:64 * par + 64, b2, h, 0:32],
                        E_t[64 * par:64 * par + 64, b2, h, :],
                        start=True, stop=True,
                        tile_position=(64 * par, 32 * hp))
                    nc.tensor.matmul(
                        rs[32 * hp:32 * hp + 32, :],
                        ones32_t[64 * par:64 * par + 64, :],
                        E_t[64 * par:64 * par + 64, b2, h, :],
                        start=True, stop=True,
                        tile_position=(64 * par, 32 * hp))
                rr = asb.tile([128, NPIX], F32, tag="rcpB",
                              name=f"rr{salt}{b2}{par}", bufs=2)
                nc.vector.reciprocal(rr[:], rs[:])
                nc.vector.tensor_mul(concat[oh][:, 2 * b2 + par, :],
                                     av[:], rr[:])

        with tc.tile_pool(name="cdata", bufs=1) as cd, \
             tc.tile_pool(name="convps", bufs=1, space="PSUM") as cvp, \
             tc.tile_pool(name="attnsb", bufs=1) as asb:
            # ---- DMA emission in deadline order.  xv rides the scalar
            # queue in parallel with wv's serial sync-queue chunks so
            # P4's first matmuls unblock as early as possible. ----
            xv = load_x(cd, 'v', nc.scalar)
            wv_ch = load_weights_resident(nc, nc.sync, wv_t, dt["wv"].ap(),
                                          chunk_plan(2, 4), par_head=6)
            nc.vector.memset(VT[:, :, :, 32:33], 1.0)
            nc.vector.memset(ones32_t[:], 1.0)
            # ---- PE warm-up: the HAM clock-gate needs ~3.4us of
            # sustained activity to lift the PE from 1.2 to 2.4 GHz.
            # The input DMAs take ~9us to land; burn that dead time on
            # throwaway matmuls so the first real conv runs warm. ----
            scratch = cd.tile([128, 512], BF16, name="scratch")
            nc.vector.memset(scratch[:], 1.0)
            warm_ps = cvp.tile([128, 512], F32, tag="cvB01",
                               name="warm_ps")
            for wi in range(22):
                nc.tensor.matmul(warm_ps[:], scratch[:, 0:128],
                                 scratch[:], start=True, stop=True)
            xq = load_x(cd, 'q', nc.gpsimd)
            xk = load_x(cd, 'k', nc.gpsimd)
            wk_ch = load_weights_resident(nc, nc.sync, wk_t, dt["wk"].ap(),
                                          chunk_plan(8, 8))
            for nmm in ("q", "k", "v"):
                nc.gpsimd.dma_start(bias_t[nmm][:],
                                    dt[f"bias{nmm}"].ap()
                                    .rearrange("h c -> c h"))
            nc.gpsimd.dma_start(bias2v_t[:],
                                dt["bias2v"].ap().rearrange("h c -> c h"))
            nc.gpsimd.dma_start(ones_t[:], dt["ones"][:])
            nc.vector.memset(ones512_t[:], 1.0)
            nc.gpsimd.dma_start(biasw_t[:], dt["bias2k"][:])
            nc.gpsimd.dma_start(bo_t[:], dt["bo"][:])
            nc.scalar.dma_start(ident_t[:], dt["ident"][:])
            for h in range(2):
                nc.scalar.dma_start(wo_tt[h][:], dt["wo_t"][h])
            x1 = {}
            for nmm in ("q", "k", "v"):
                x1[nmm] = cd.tile([128, 2, 8, 8, BL], FP8, name=f"x1{nmm}t")

            # ---- P4: v -> v1 (desc order = storage/DMA order) ----
            conv_pass(nc, cvp, wv_ch, [xv], tag="p4", ptags=["cvA"],
                      on_bank_done=relu_hook(x1['v'], bias_t['v']))
            # ---- P5: v1 -> vh (asc order: reads rows 0-3 first) ----
            conv_pass(nc, cvp, wv_ch, [x1['v']], tag="p5", ptags=["cvB"],
                      order=ASC, on_bank_done=vh_hook(hh['v'], bias2v_t))

            # ---- V transposes: vh [c,b,pix] -> VT [kpix, b2, h, dk|1] ----
            # interleaved into P1's matmul stream to keep HAM warm;
            # skipped for the first taps so P5's drain ACTs clear first
            vt_jobs = []
            for b in range(BL):
                par, b2 = b % 2, b // 2
                for oh in range(2):
                    vt_jobs.append((b, par, b2, oh))
            vt_state = {'i': 0}

            def emit_vt(njobs=6):
                for _ in range(njobs):
                    i = vt_state['i']
                    if i >= len(vt_jobs):
                        return
                    vt_state['i'] = i + 1
                    b, par, b2, oh = vt_jobs[i]
                    pvt = cvp.tile([64, 128], BF16, tag=f"cvB0{i % 2}",
                                   name=f"pvt{b}{oh}")
                    nc.tensor.transpose(pvt[:], hh['v'][oh][:, b, :],
                                        ident_t[:])
                    nc.vector.tensor_copy(
                        VT[64 * par:64 * par + 64, b2,
                           oh * 4:(oh + 1) * 4, 0:32],
                        pvt[:].rearrange("k (h d) -> k h d", h=4))

            # ---- P1: q -> q1 (stream wq, storage order) ----
            with tc.tile_pool(name="wstream", bufs=3) as wsp:
                wq_ch = load_weights_stream(nc, wsp, dt["wq"].ap(), tag="q")
                conv_pass(nc, cvp, wq_ch, [xq], tag="p1", ptags=["cvA"],
                          interleave=emit_vt, interleave_skip=8,
                          on_bank_done=relu_hook(x1['q'], bias_t['q']))
                emit_vt(len(vt_jobs))    # any leftovers

            # ---- P2: k -> k1 (desc) ----
            conv_pass(nc, cvp, wk_ch, [xk], tag="p2", ptags=["cvA"],
                      on_bank_done=relu_hook(x1['k'], bias_t['k']))

            # ---- P3a: {k1, q1} -> kh/qh oh=0 half (asc, shared LS) ----
            conv_pass(nc, cvp, wk_ch, [x1['k'], x1['q']], tag="p3a",
                      ptags=["cvA", "cvB"], order=ASC, oh_list=(0,),
                      finish=False, on_bank_done=kq_hook)

            # ---- P3b: oh=1 half, with heads 0-3's attention chain
            # interleaved between its conv matmuls (they only need the
            # oh=0 outputs + VT).  Psum rides the freed oh=0 tag rings.
            attn_jobs = []
            for s in range(BL // 2 + 2):
                if s < BL // 2:
                    attn_jobs.append(('sc', s))
                if 2 <= s:
                    attn_jobs.append(('av', s - 2))
            aj_state = {'i': 0}

            def emit_attn_a(njobs=1):
                for _ in range(njobs):
                    i = aj_state['i']
                    if i >= len(attn_jobs):
                        return
                    aj_state['i'] = i + 1
                    op, b2 = attn_jobs[i]
                    if op == 'sc':
                        emit_scores_h(cvp, ["cvA00", "cvA01"], b2, 0, 'a')
                    else:
                        emit_av_sw(b2, 0, ["cvB00"], ["cvB01"], 'a')

            conv_pass(nc, cvp, wk_ch, [x1['k'], x1['q']], tag="p3b",
                      ptags=["cvA", "cvB"], order=ASC, oh_list=(1,),
                      finish=False, on_bank_done=kq_hook,
                      interleave=emit_attn_a, interleave_skip=4,
                      interleave_every=3)
            emit_attn_a(len(attn_jobs))    # flush leftovers

            # ---- attention tail: heads 4-7 + projection, pipelined
            # per batch-pair: sc(b2) | av(b2-1) | proj(b2-2).  AV here
            # runs V-stationary: out[dk, qpix] lands [c, pix]-shaped (4
            # heads packed via col groups), a ones-stationary matmul
            # replicates each head's rowsum across its dk partitions,
            # and one reciprocal + one elementwise multiply writes
            # concat directly — no O-transposes.

            def emit_proj(blk):
                pspr = cvp.tile([128, 256], F32, tag="cvB00",
                                name=f"pspr{blk}",
                                padded_shape=[128, 512])
                for oh in range(2):
                    nc.tensor.matmul(
                        pspr[:],
                        concat[oh].rearrange("c b p -> c (b p)")
                        [:, blk * 128:(blk + 1) * 128],
                        wo_tt[oh][:], start=(oh == 0), stop=False)
                nc.tensor.matmul(pspr[:], ones_t[:], bo_t[:],
                                 start=False, stop=True)
                osb = asb.tile([128, 256], F32, tag="osb",
                               name=f"osb{blk}", bufs=2)
                nc.vector.tensor_copy(osb[:], pspr[:])
                nc.sync.dma_start(dt["out"][blk], osb[:])

            for step in range(BL // 2 + 2):
                if step < BL // 2:
                    emit_scores_h(cvp, ["cvA00", "cvA01"], step, 1, 'b')
                if 1 <= step < BL // 2 + 1:
                    emit_av_sw(step - 1, 1, ["cvA10", "cvA11"],
                               ["cvB10", "cvB11"], 'b')
                if 2 <= step:
                    emit_proj(step - 2)
    nc.compile()
    return nc


# ---------------------------------------------------------------------------
# Host-side prep
# ---------------------------------------------------------------------------

def prep_weights(w):
    """w: [D, D, 15, 15] OIHW -> [128, NTAPS, 2, 256] fp8e4 laid out
    (c_lo, tap, c-half, o), scaled by WS."""
    wt = np.empty((NTAPS, 2, 128, 256), np.float32)
    for i, (sr, sc, *_r) in enumerate(TAPS):
        # [O, I] -> [I, O] -> [ch, c_lo, O]
        wt[i] = (w[:, :, sr + 7, sc + 7].T * WS).reshape(2, 128, 256)
    wt = wt.transpose(2, 0, 1, 3)   # -> [c_lo, tap, ch, o]
    return np.ascontiguousarray(wt).astype(ml_dtypes.float8_e4m3)


def prep_static(wk, bk, wq, bq, wv, bv, wo, bo):
    """Host-side weight prep shared by all cores."""
    st = {}
    for nmm, w, b in (("q", wq, bq), ("k", wk, bk), ("v", wv, bv)):
        st[f"w{nmm}"] = prep_weights(np.asarray(w, np.float32))
        st[f"bias{nmm}"] = np.ascontiguousarray(
            (np.asarray(b, np.float32) * AS).reshape(2, 128))
    st["bias2v"] = np.ascontiguousarray(
        np.asarray(bv, np.float32).reshape(2, 128))
    st["wo_t"] = np.ascontiguousarray(
        np.asarray(wo, np.float32).T).reshape(2, 128, 256).astype(
        ml_dtypes.bfloat16)
    st["bo"] = np.asarray(bo, np.float32).reshape(1, 256).astype(
        ml_dtypes.bfloat16)
    st["ones"] = np.ones((1, 128), ml_dtypes.bfloat16)
    st["bias2k"] = (np.asarray(bk, np.float32) * F2).reshape(1, 256).astype(
        ml_dtypes.bfloat16)
    st["ident"] = np.eye(128, dtype=ml_dtypes.bfloat16)
    return st


def prep_core_x(x, core):
    """x: [B, 8, 8, D] -> this core's [128, 2, 8, 8, BL] fp8
    (c_lo, c-half, row, col, b), scaled by XS."""
    xs = np.asarray(x[core * BL:(core + 1) * BL], np.float32) * XS
    xs = xs.transpose(3, 1, 2, 0)                    # [D, r, c, b]
    xs = xs.reshape(2, 128, 8, 8, BL).transpose(1, 0, 2, 3, 4)
    return np.ascontiguousarray(np.clip(xs, -240, 240)).astype(
        ml_dtypes.float8_e4m3)


def make_in_maps(q, k, v, st):
    in_maps = []
    for core in range(NCORES):
        m = dict(st)
        m["xq"] = prep_core_x(q, core)
        m["xk"] = prep_core_x(k, core)
        m["xv"] = prep_core_x(v, core)
        in_maps.append(m)
    return in_maps


def gather_out(results):
    """results: list of dicts with 'out' [8, 128, 256] -> [B, 8, 8, D]."""
    outs = [r["out"].reshape(BL, 8, 8, D) for r in results]
    return np.concatenate(outs, axis=0)


# ---------------------------------------------------------------------------
# Self-contained entry point: kernel(**inputs) -> full [128, 8, 8, 256]
# ---------------------------------------------------------------------------
_NC_CACHE = None


def _get_nc():
    global _NC_CACHE
    if _NC_CACHE is None:
        _NC_CACHE = build_kernel()
    return _NC_CACHE


def kernel(q, k, v, wk, bk, wq, bq, wv, bv, wo, bo):
    nc = _get_nc()
    st = prep_static(wk, bk, wq, bq, wv, bv, wo, bo)
    in_maps = make_in_maps(np.asarray(q), np.asarray(k), np.asarray(v), st)
    res = bass_utils.run_bass_kernel_spmd(
        nc, in_maps, core_ids=list(range(NCORES)))
    return gather_out(res.results)
